# revision 36
# baseline (speedup 1.0000x reference)
"""Trainium2 Bass kernel for nn_Encoder (tree GNN message passing).

Data-parallel over batch: 8 cores x 4 batch items. Feature-major layout
(features on partitions, (batch, node) on the free dim). The per-node
direction select is folded into the matmul: activations are block-stacked
into Htilde (K = 7*2*idim rows + 7 bias rows) whose rows are ordered
[even-sourced region | odd-sourced region | one-hot bias rows], so each
128-row K-tile is built with 1-2 full-width fused DVE ops
  out = (vec_rep == tcol) * child
with tcol a per-partition scalar column. Weights are host-reordered to
match (dmap/drev baked in); the per-node bias bg[dmap[t]] rides as 7 extra
K-rows whose rhs is the one-hot of vec. Outputs use M-replication (weights
tiled along M) so layers with odim<128 still fill all 128 partitions —
which also makes X replicated with period odim, which in turn makes every
partition range of X a valid child operand. PReLU = Relu(z) - a*Relu(-z)
(2 ACT passes + 1 fused DVE op). fp32r matmuls for leaf/k<=7, exact fp32
for k>=8 (same speed at their column counts).
"""

import numpy as np
from contextlib import ExitStack

import concourse.bass as bass
import concourse.bacc as bacc
import concourse.tile as tile
from concourse import mybir
from concourse.bass_utils import run_bass_kernel_spmd

# ----- problem constants (hardcoded per harness contract) -----
B = 32
N = 8192
NLAYERS = 14
S = 3                      # SAMPLE_LAYERS
NDIR = 7
NCORES = 8
BPC = B // NCORES          # batches per core = 4

ODIMS = [8]
for _ in range(1, NLAYERS):
    ODIMS.append(min(ODIMS[-1] * 2, 512))

F32 = mybir.dt.float32
F32R = mybir.dt.float32r
U8 = mybir.dt.uint8
RELU = mybir.ActivationFunctionType.Relu
MULT = mybir.AluOpType.mult
ADD = mybir.AluOpType.add
ISEQ = mybir.AluOpType.is_equal

# per-layer matmul dtype: True -> fp32r (fast), False -> fp32 (exact)
USE_F32R = {k: True for k in range(NLAYERS)}
TAIL0 = 8                 # first node-major (weight-streaming) layer

NBLK = 16                        # front-end blocks (quarter-batches)
LEAF_COLS = BPC * N // NBLK      # 4096 leaf cols per block


def _cfg(k):
    idim, odim = ODIMS[k - 1], ODIMS[k]
    n = N >> k
    rep = max(1, 128 // odim)
    modim = odim * rep
    mtiles = modim // 128
    reg_o = ((NDIR * idim + 31) // 32) * 32   # odd region start (32-aligned)
    bias0 = reg_o + NDIR * idim               # bias rows start
    krows = bias0 + NDIR                      # total K rows
    nkt = (krows + 127) // 128
    c = dict(k=k, idim=idim, odim=odim, n=n, rep=rep, modim=modim,
             mtiles=mtiles, krows=krows, nkt=nkt, reg_o=reg_o, bias0=bias0)
    if k >= S:
        sdim = ODIMS[k - S]
        fk = max(1, odim // 128)
        fs = max(1, sdim // 128)
        # Ws chain segments: x-chunks then sg-chunks, each its own K-tile
        segs = [(f, min(128, odim - f * 128), "x") for f in range(fk)]
        segs += [(f, min(128, sdim - f * 128), "s") for f in range(fs)]
        c.update(sdim=sdim, fk=fk, fs=fs, segs=segs)
    return c


def _frags(k):
    """STT build fragments per K-tile: (kt, r0, span, region, sub).
    Row layout: e-region [0, 7*idim), pad, o-region [reg_o, reg_o+7*idim),
    bias rows [bias0, bias0+7). Within a region: t = off // idim,
    feature = off % idim, sub = feature // 128. All fragment starts are
    32-aligned (hardware partition-offset granularity)."""
    c = _cfg(k)
    idim = c["idim"]
    # regions extended to cover padding/bias rows (tcol=99 there -> writes 0,
    # later overwritten by the bias DMA where applicable)
    regions = [(0, 0, c["reg_o"]), (1, c["reg_o"], c["nkt"] * 128)]
    out = []
    for kt in range((c["bias0"] + 127) // 128):
        lo = kt * 128
        hi = min(lo + 128, ((c["krows"] + 31) // 32) * 32)
        for region, ra, rb in regions:
            a, bnd = max(lo, ra), min(hi, rb)
            if a >= bnd:
                continue
            off = a - ra
            sub = (off % idim) // 128
            out.append((kt, a - lo, bnd - a, region, sub))
    return out




def _host_prep(points, dmap, drev, vecs, params):
    dmap = np.asarray(dmap).astype(np.int64)
    drev = np.asarray(drev).astype(np.int64)
    pl = params["leaf"]
    leafW = np.tile(np.asarray(pl["W"], np.float32), (1, 16))        # [3,128]
    leafb = np.tile(np.asarray(pl["b"], np.float32), 16)[:, None]    # [128,1]
    alphas = {0: float(pl["a"])}

    wg_blobs, ws_blobs, bs_blobs, tcols = {}, {}, {}, {}
    wgs_blobs, bgm_blobs = {}, {}
    for k in range(1, NLAYERS):
        c = _cfg(k)
        p = params["layers"][k - 1]
        alphas[k] = float(p["a"])
        Wg = np.asarray(p["Wg"], np.float32)   # [7, 2idim, odim]
        bg = np.asarray(p["bg"], np.float32)   # [7, odim]
        idim = c["idim"]
        blob = np.zeros((c["nkt"] * 128, c["modim"]), np.float32)
        tc_ = np.full((c["nkt"] * 128,), 99.0, np.float32)
        for r in range(2):
            for t in range(NDIR):
                hf = int(drev[t]) ^ r
                g0 = r * c["reg_o"] + t * idim
                blob[g0:g0 + idim, :] = np.tile(
                    Wg[dmap[t]][hf * idim:(hf + 1) * idim], (1, c["rep"]))
                tc_[g0:g0 + idim] = t
        for t in range(NDIR):
            blob[c["bias0"] + t, :] = np.tile(bg[dmap[t]], c["rep"])
        wg_blobs[k] = blob.reshape(c["nkt"], 128, c["modim"])
        tcols[k] = tc_.reshape(c["nkt"], 128).T.copy()   # [128, nkt]
        if k >= TAIL0:
            # streaming layout, partition-major: [7, 128, nsub*odim] so each
            # direction's weights DMA as one transfer with 16KB/partition runs
            wgs_blobs[k] = np.ascontiguousarray(
                np.stack([Wg[dmap[t]] for t in range(NDIR)]).reshape(
                    NDIR, 2 * idim // 128, 128, c["odim"]).transpose(0, 2, 1, 3)
                .reshape(NDIR, 128, (2 * idim // 128) * c["odim"]))
            bgm_blobs[k] = np.stack([bg[dmap[t]] for t in range(NDIR)])  # [7, odim]
        if k >= S:
            Ws = np.tile(np.asarray(p["Ws"], np.float32), (1, c["rep"]))
            bs = np.asarray(p["bs"], np.float32)
            blob2 = np.zeros((len(c["segs"]), 128, c["modim"]), np.float32)
            r0 = 0
            for i, (f, rows, _src) in enumerate(c["segs"]):
                blob2[i, :rows, :] = Ws[r0:r0 + rows]
                r0 += rows
            ws_blobs[k] = blob2
            bs_blobs[k] = np.tile(bs, c["rep"])[:, None]   # [modim, 1]

    in_maps = []
    for core in range(NCORES):
        bsl = slice(core * BPC, (core + 1) * BPC)
        m = {
            "pts": np.ascontiguousarray(
                np.asarray(points[bsl], np.float32).reshape(BPC * N, 3).T),
            "leafW": leafW, "leafb": leafb, "leafnb": -leafb,
        }
        for k in range(1, NLAYERS):
            c = _cfg(k)
            v = np.asarray(vecs[k - 1][bsl], np.int64).reshape(1, -1)
            if k < TAIL0:
                m[f"vec{k}"] = v.astype(np.uint8)
                oh = np.zeros((NDIR, BPC * c["n"]), np.float32)
                oh[v[0], np.arange(BPC * c["n"])] = 1.0
                m[f"oh{k}"] = oh
                m[f"wg{k}"] = wg_blobs[k]
                m[f"tc{k}"] = tcols[k]
            else:
                m[f"rrep{k}"] = drev[v].astype(np.uint8)        # [1, cols]
                m[f"vcol{k}"] = v.astype(np.float32).T          # [cols, 1]
                m[f"wgs{k}"] = wgs_blobs[k]
                m[f"bgm{k}"] = bgm_blobs[k]
            if k >= S:
                m[f"ws{k}"] = ws_blobs[k]
                m[f"bs{k}"] = bs_blobs[k]
                m[f"nbs{k}"] = -bs_blobs[k]
        in_maps.append(m)
    return in_maps, alphas, drev


def _build(nc, alphas):
    def rr(ap, k):
        return ap.bitcast(F32R) if USE_F32R[k] else ap

    d = {}
    d["pts"] = nc.declare_dram_parameter("pts", [3, BPC * N], F32, isOutput=False)
    d["leafW"] = nc.declare_dram_parameter("leafW", [3, 128], F32, isOutput=False)
    d["leafb"] = nc.declare_dram_parameter("leafb", [128, 1], F32, isOutput=False)
    d["leafnb"] = nc.declare_dram_parameter("leafnb", [128, 1], F32, isOutput=False)
    for k in range(1, NLAYERS):
        c = _cfg(k)
        cols = BPC * c["n"]
        if k < TAIL0:
            d[f"vec{k}"] = nc.declare_dram_parameter(f"vec{k}", [1, cols], U8, isOutput=False)
            d[f"oh{k}"] = nc.declare_dram_parameter(f"oh{k}", [NDIR, cols], F32, isOutput=False)
            d[f"wg{k}"] = nc.declare_dram_parameter(
                f"wg{k}", [c["nkt"], 128, c["modim"]], F32, isOutput=False)
            d[f"tc{k}"] = nc.declare_dram_parameter(f"tc{k}", [128, c["nkt"]], F32, isOutput=False)
        else:
            d[f"rrep{k}"] = nc.declare_dram_parameter(f"rrep{k}", [1, cols], U8, isOutput=False)
            d[f"vcol{k}"] = nc.declare_dram_parameter(f"vcol{k}", [cols, 1], F32, isOutput=False)
            d[f"wgs{k}"] = nc.declare_dram_parameter(
                f"wgs{k}", [NDIR, 128, (2 * c["idim"] // 128) * c["odim"]], F32, isOutput=False)
            d[f"bgm{k}"] = nc.declare_dram_parameter(f"bgm{k}", [NDIR, c["odim"]], F32, isOutput=False)
        if k >= S:
            d[f"ws{k}"] = nc.declare_dram_parameter(
                f"ws{k}", [len(c["segs"]), 128, c["modim"]], F32, isOutput=False)
            d[f"bs{k}"] = nc.declare_dram_parameter(f"bs{k}", [c["modim"], 1], F32, isOutput=False)
            d[f"nbs{k}"] = nc.declare_dram_parameter(f"nbs{k}", [c["modim"], 1], F32, isOutput=False)
    d["out"] = nc.declare_dram_parameter("out", [512, BPC], F32, isOutput=True)

    with tile.TileContext(nc) as tc, ExitStack() as ctx:
        persist = ctx.enter_context(tc.tile_pool(name="persist", bufs=1))
        layerbuf = ctx.enter_context(tc.tile_pool(name="layerbuf", bufs=1))
        layer2 = ctx.enter_context(tc.tile_pool(name="layer2", bufs=2))
        htp = ctx.enter_context(tc.tile_pool(name="ht", bufs=2))
        wsp = ctx.enter_context(tc.tile_pool(name="ws", bufs=2))
        pp = ctx.enter_context(tc.tile_pool(name="ps", bufs=8, space="PSUM"))
        tp = ctx.enter_context(tc.tile_pool(name="tmp", bufs=2))

        t_leafW = persist.tile([3, 128], F32, tag="leafW")
        t_leafb = persist.tile([128, 1], F32, tag="leafb")
        t_leafnb = persist.tile([128, 1], F32, tag="leafnb")
        nc.sync.dma_start(out=rr(t_leafW[:], 0), in_=rr(d["leafW"].ap(), 0))
        nc.sync.dma_start(out=t_leafb, in_=d["leafb"].ap())
        nc.sync.dma_start(out=t_leafnb, in_=d["leafnb"].ap())
        t_bs, t_nbs, t_tc = {}, {}, {}
        for k in range(1, NLAYERS):
            c = _cfg(k)
            if k < TAIL0:
                t_tc[k] = persist.tile([128, c["nkt"]], F32, tag=f"tc{k}", name=f"tc{k}")
                nc.sync.dma_start(out=t_tc[k], in_=d[f"tc{k}"].ap())
            if k >= S:
                t_bs[k] = persist.tile([128, c["mtiles"]], F32, tag=f"bs{k}", name=f"bs{k}")
                t_nbs[k] = persist.tile([128, c["mtiles"]], F32, tag=f"nbs{k}", name=f"nbs{k}")
                nc.sync.dma_start(out=t_bs[k], in_=d[f"bs{k}"].ap().rearrange(
                    "(m p) o -> p (m o)", p=128))
                nc.sync.dma_start(out=t_nbs[k], in_=d[f"nbs{k}"].ap().rearrange(
                    "(m p) o -> p (m o)", p=128))

        from concourse.masks import make_identity
        t_ident = persist.tile([128, 128], F32, tag="ident")
        make_identity(nc, t_ident[:])
        t_ones = persist.tile([1, 128], F32, tag="ones")
        nc.vector.memset(t_ones[:], 1.0)

        X = {}
        for k in range(S, NLAYERS):
            c = _cfg(k)
            X[k] = persist.tile([128, max(1, c["odim"] // 128), BPC, c["n"]],
                                F32, tag=f"X{k}", name=f"X{k}")
        sg = {}
        for k in range(S + 1, NLAYERS):
            c = _cfg(k)
            sg[k] = persist.tile([128, c["fs"], BPC, c["n"]], F32, tag=f"sg{k}", name=f"sg{k}")

        def prelu_store(ps_ap, out_ap, alpha, bias=None, nbias=None, eng=None):
            # prelu(z+b) = pos + a*((z+b) - pos), pos = relu(z+b)
            # 1 ACT pass + 2 DVE ops
            shape = [ps_ap.partition_size(), ps_ap.free_size()]
            pos = tp.tile(shape, F32, tag="pos")
            neg = tp.tile(shape, F32, tag="neg")
            nc.scalar.activation(pos[:], ps_ap, RELU,
                                 bias=(bias if bias is not None else 0.0), scale=1.0)
            nc.vector.scalar_tensor_tensor(
                out=neg[:], in0=ps_ap,
                scalar=(bias if bias is not None else 0.0), in1=pos[:],
                op0=ADD, op1=mybir.AluOpType.subtract)
            nc.vector.scalar_tensor_tensor(
                out=out_ap, in0=neg[:], scalar=alpha, in1=pos[:],
                op0=MULT, op1=ADD)

        # ------------ front-end: leaf + k=1..3, pairs of blocks stage-major ------------
        # Front-end pools live only for this phase; their SBUF is returned
        # before the deep back-end weight-prefetch pool opens.
        fe_ctx = ExitStack()
        fe2 = fe_ctx.enter_context(tc.tile_pool(name="fe2", bufs=2))
        wp = fe_ctx.enter_context(tc.tile_pool(name="fwp", bufs=4))
        BLKB = NBLK // BPC               # blocks per batch item
        X0bs, xprevs = {}, {}

        def fe_leaf(blk):
            X0b = fe2.tile([128, LEAF_COLS], F32, tag="X0b", name=f"X0b{blk}")
            for cc in range(LEAF_COLS // 512):
                t_pts = fe2.tile([3, 512], F32, tag="pts", name=f"pts{blk}_{cc}")
                nc.sync.dma_start(
                    out=rr(t_pts[:], 0),
                    in_=rr(d["pts"].ap()[:, blk * LEAF_COLS + cc * 512:
                                         blk * LEAF_COLS + (cc + 1) * 512], 0))
                ps = pp.tile([128, 512], F32, tag="ps", name=f"lps{blk}_{cc}")
                nc.tensor.matmul(ps[:], rr(t_leafW[:], 0), rr(t_pts[:], 0),
                                 start=True, stop=True)
                prelu_store(ps[:], rr(X0b[:, cc * 512:(cc + 1) * 512], S),
                            alphas[0], bias=t_leafb[:], nbias=t_leafnb[:])
            X0bs[blk] = X0b
            xprevs[blk] = X0b

        def fe_layer(blk, k):
            b, half = blk // BLKB, blk % BLKB
            X0b = X0bs[blk]
            xb_prev = xprevs[blk]
            c = _cfg(k)
            idim = c["idim"]
            cols = LEAF_COLS >> k
            col0 = blk * cols
            vrep = fe2.tile([128, cols], U8, tag="vrep", name=f"vrep{blk}_{k}")
            vap = d[f"vec{k}"].ap()
            nc.sync.dma_start(out=vrep, in_=bass.AP(
                tensor=vap.tensor, offset=vap.offset + col0,
                ap=[[0, 128], [1, cols]]))
            ht = fe2.tile([128, c["nkt"], cols], F32, tag="fht", name=f"fht{blk}_{k}")
            for kt, r0, span, region, sub in _frags(k):
                nc.vector.scalar_tensor_tensor(
                    out=rr(ht[r0:r0 + span, kt, :], k),
                    in0=vrep[r0:r0 + span, :],
                    scalar=t_tc[k][r0:r0 + span, kt:kt + 1],
                    in1=xb_prev[r0:r0 + span, region::2],
                    op0=ISEQ, op1=MULT)
            bkt, br0 = divmod(c["bias0"], 128)
            nc.sync.dma_start(
                out=rr(ht[br0:br0 + NDIR, bkt, :], k),
                in_=rr(d[f"oh{k}"].ap()[:, col0:col0 + cols], k))
            xm = fe2.tile([128, cols], F32, tag=f"xm{k}", name=f"xm{blk}_{k}")
            nccs = max(1, cols // 512)
            ccw = min(512, cols)
            pss = [pp.tile([128, ccw], F32, tag="ps", name=f"fps{blk}_{k}_{i}")
                   for i in range(nccs)]
            for kt in range(c["nkt"]):
                rows = min(128, c["krows"] - kt * 128)
                wt = wp.tile([128, c["modim"]], F32, tag="wt")
                nc.sync.dma_start(out=rr(wt[:], k), in_=rr(d[f"wg{k}"].ap()[kt], k))
                for cc in range(nccs):
                    nc.tensor.matmul(
                        pss[cc][:], rr(wt[0:rows, 0:128], k),
                        rr(ht[0:rows, kt, cc * ccw:(cc + 1) * ccw], k),
                        start=(kt == 0), stop=(kt == c["nkt"] - 1))
            if k < S:
                for cc in range(nccs):
                    prelu_store(pss[cc][:], xm[:, cc * ccw:(cc + 1) * ccw],
                                alphas[k])
                kc = k + S
                scols = cols // 8
                nc.vector.tensor_copy(
                    rr(sg[kc][:, 0, b, half * scols:(half + 1) * scols], kc),
                    xm[:, ::8])
                xprevs[blk] = xm
            else:
                for cc in range(nccs):
                    prelu_store(pss[cc][:], rr(xm[:, cc * ccw:(cc + 1) * ccw], k),
                                alphas[k])
                ps2 = pp.tile([128, cols], F32, tag="ps", name=f"wps{blk}")
                for ci, (f, rows, src) in enumerate(c["segs"]):
                    wt2 = wsp.tile([128, c["modim"]], F32, tag="wst")
                    nc.sync.dma_start(out=rr(wt2[:], k),
                                      in_=rr(d[f"ws{k}"].ap()[ci], k))
                    rhs = (xm[0:rows, :] if src == "x"
                           else X0b[0:rows, ::8])
                    nc.tensor.matmul(ps2[:], rr(wt2[0:rows, 0:128], k),
                                     rr(rhs, k), start=(ci == 0),
                                     stop=(ci == len(c["segs"]) - 1))
                prelu_store(ps2[:],
                            X[S][:, 0, b, half * cols:(half + 1) * cols],
                            alphas[k], bias=t_bs[k][:, 0:1], nbias=t_nbs[k][:, 0:1])

        for pb in range(0, NBLK, 2):
            pair = (pb, pb + 1)
            for blk in pair:
                fe_leaf(blk)
            for k in range(1, S + 1):
                for blk in pair:
                    fe_layer(blk, k)
        fe_ctx.close()
        wp = ctx.enter_context(tc.tile_pool(name="w", bufs=8))

        # sg for layer 6 comes from the completed X[3]
        nc.vector.tensor_copy(rr(sg[6][:, 0, :, :], 6), X[S][:, 0, :, ::8])

        # ---------------- back-end (feature-major): k = 4..TAIL0-1 ----------------
        for k in range(S + 1, TAIL0):
            c = _cfg(k)
            n, idim, odim = c["n"], c["idim"], c["odim"]
            cols = BPC * n
            vrep = layer2.tile([128, BPC, n], U8, tag="vrepB")
            vap = d[f"vec{k}"].ap()
            nc.sync.dma_start(out=vrep, in_=bass.AP(
                tensor=vap.tensor, offset=vap.offset,
                ap=[[0, 128], [n, BPC], [1, n]]))
            t_oh = layerbuf.tile([NDIR, BPC, n], F32, tag="ohB")
            nc.sync.dma_start(
                out=rr(t_oh[:], k),
                in_=rr(d[f"oh{k}"].ap().rearrange("p (b n) -> p b n", b=BPC), k))

            nccs = max(1, cols // 512)
            ccw = min(512, cols)
            pss = {(m, cc): pp.tile([128, ccw], F32, tag="ps", name=f"ps{k}_{m}_{cc}")
                   for m in range(c["mtiles"]) for cc in range(nccs)}
            xprev = X[k - 1]
            frags = _frags(k)
            nht = c["nkt"] - 1           # Htilde K-tiles (last tile is bias-only)
            gkt = max(1, 1024 // cols)   # K-tiles per ht group (~4KB)
            for kt0 in range(0, nht, gkt):
                kts = range(kt0, min(kt0 + gkt, nht))
                ht = htp.tile([128, len(kts), BPC, n], F32, tag="bht")
                for kt, r0, span, region, sub in frags:
                    if kt not in kts:
                        continue
                    nc.vector.scalar_tensor_tensor(
                        out=rr(ht[r0:r0 + span, kt - kt0, :, :], k),
                        in0=vrep[r0:r0 + span, :, :],
                        scalar=t_tc[k][r0:r0 + span, kt:kt + 1],
                        in1=xprev[r0:r0 + span, sub, :, region::2],
                        op0=ISEQ, op1=MULT)
                for kt in kts:
                    wt = wp.tile([128, c["modim"]], F32, tag="wt")
                    nc.sync.dma_start(out=rr(wt[:], k), in_=rr(d[f"wg{k}"].ap()[kt], k))
                    rhs2d = ht[:, kt - kt0, :, :].rearrange("p b n -> p (b n)")
                    for m in range(c["mtiles"]):
                        for cc in range(nccs):
                            nc.tensor.matmul(
                                pss[(m, cc)][:],
                                rr(wt[:, m * 128:(m + 1) * 128], k),
                                rr(rhs2d[:, cc * ccw:(cc + 1) * ccw], k),
                                start=(kt == 0), stop=False)
            wt = wp.tile([128, c["modim"]], F32, tag="wt")
            nc.sync.dma_start(out=rr(wt[:], k), in_=rr(d[f"wg{k}"].ap()[c["nkt"] - 1], k))
            oh2d = t_oh[:].rearrange("p b n -> p (b n)")
            for m in range(c["mtiles"]):
                for cc in range(nccs):
                    nc.tensor.matmul(
                        pss[(m, cc)][:], rr(wt[0:NDIR, m * 128:(m + 1) * 128], k),
                        rr(oh2d[:, cc * ccw:(cc + 1) * ccw], k),
                        start=False, stop=True)
            xm = layerbuf.tile([128, c["mtiles"], BPC, n], F32, tag="xmB")
            for m in range(c["mtiles"]):
                xm2d = xm[:, m, :, :].rearrange("p b n -> p (b n)")
                for cc in range(nccs):
                    prelu_store(pss[(m, cc)][:],
                                rr(xm2d[:, cc * ccw:(cc + 1) * ccw], k), alphas[k])
            pss2 = {(m, cc): pp.tile([128, ccw], F32, tag="ps", name=f"ps2_{k}_{m}_{cc}")
                    for m in range(c["mtiles"]) for cc in range(nccs)}
            for ci, (f, rows, src) in enumerate(c["segs"]):
                wt2 = wsp.tile([128, c["modim"]], F32, tag="wst")
                nc.sync.dma_start(out=rr(wt2[:], k), in_=rr(d[f"ws{k}"].ap()[ci], k))
                rhs2d = (xm[0:rows, f, :, :] if src == "x"
                         else sg[k][0:rows, f, :, :]).rearrange("p b n -> p (b n)")
                for m in range(c["mtiles"]):
                    for cc in range(nccs):
                        nc.tensor.matmul(
                            pss2[(m, cc)][:],
                            rr(wt2[0:rows, m * 128:(m + 1) * 128], k),
                            rr(rhs2d[:, cc * ccw:(cc + 1) * ccw], k),
                            start=(ci == 0), stop=(ci == len(c["segs"]) - 1))
            for m in range(c["mtiles"]):
                x2d = X[k][:, m, :, :].rearrange("p b n -> p (b n)")
                for cc in range(nccs):
                    prelu_store(pss2[(m, cc)][:], x2d[:, cc * ccw:(cc + 1) * ccw],
                                alphas[k], bias=t_bs[k][:, m:m + 1],
                                nbias=t_nbs[k][:, m:m + 1])
            kc = k + S
            if kc < NLAYERS:
                for f in range(max(1, odim // 128)):
                    nc.vector.tensor_copy(
                        rr(sg[kc][:, f, :, :], kc), X[k][:, f, :, ::8])

        # ---------------- tail (node-major, weight-streaming): k >= TAIL0 ----------------
        for k in range(TAIL0, NLAYERS):
            c = _cfg(k)
            n, idim, odim = c["n"], c["idim"], c["odim"]
            cols = BPC * n
            nsub = 2 * idim // 128          # ht K-tiles (8)
            xprev = X[k - 1]
            fprev = ODIMS[k - 1] // 128     # 4

            rrep = layer2.tile([128, BPC, n], U8, tag="rrepT", bufs=1)
            rap = d[f"rrep{k}"].ap()
            nc.sync.dma_start(out=rrep, in_=bass.AP(
                tensor=rap.tensor, offset=rap.offset,
                ap=[[0, 128], [n, BPC], [1, n]]))
            vcol = layer2.tile([128, 1], F32, tag="vcolT")
            nc.sync.dma_start(out=vcol[0:cols, :], in_=d[f"vcol{k}"].ap())
            mcol = layer2.tile([128, NDIR], F32, tag="mcolT")
            for t in range(NDIR):
                nc.vector.tensor_scalar(
                    out=mcol[0:cols, t:t + 1], in0=vcol[0:cols, :],
                    scalar1=float(t), scalar2=None, op0=ISEQ)

            # ht: [128, nsub, BPC, n]; rows of sub j = features of chunk j%4,
            # half j//4. top half: r=0 -> even child; bottom: r=0 -> odd.
            ht = htp.tile([128, nsub, BPC, n], F32, tag="tht")
            rrep4 = rrep[:].unsqueeze(1).broadcast_to([128, fprev, BPC, n])
            for half in range(2):
                ev = xprev[:, :, :, 0::2]
                od = xprev[:, :, :, 1::2]
                a_src = ev if half == 0 else od   # used when r == 0
                b_src = od if half == 0 else ev   # used when r == 1
                tmp1 = tp.tile([128, fprev, BPC, n], F32, tag="sel1", bufs=1)
                tmp2 = tp.tile([128, fprev, BPC, n], F32, tag="sel2", bufs=1)
                nc.vector.scalar_tensor_tensor(
                    out=tmp1[:], in0=rrep4, scalar=0.0, in1=a_src,
                    op0=ISEQ, op1=MULT)
                nc.vector.scalar_tensor_tensor(
                    out=tmp2[:], in0=rrep4, scalar=1.0, in1=b_src,
                    op0=ISEQ, op1=MULT)
                nc.vector.tensor_add(
                    rr(ht[:, half * fprev:(half + 1) * fprev, :, :], k),
                    tmp1[:], tmp2[:])

            # 7 direction matmuls: stationary ht tiles, streamed weights
            pst = {}
            for t in range(NDIR):
                pst[t] = pp.tile([128, odim], F32, tag="ps", name=f"pst{k}_{t}")
                wt8 = wp.tile([128, nsub, odim], F32, tag="wt8", bufs=2)
                nc.sync.dma_start(
                    out=rr(wt8[:].rearrange("p j o -> p (j o)"), k),
                    in_=rr(d[f"wgs{k}"].ap()[t], k))
                for j in range(nsub):
                    nc.tensor.matmul(
                        pst[t][0:cols, :],
                        rr(ht[:, j, :, :].rearrange("p b n -> p (b n)"), k),
                        rr(wt8[:, j, :], k), start=(j == 0), stop=False)
                # bias: ones-row x bg[dmap[t]]
                wb = wsp.tile([1, odim], F32, tag="wbT", bufs=1)
                nc.sync.dma_start(out=rr(wb[:], k), in_=rr(d[f"bgm{k}"].ap()[t:t + 1, :], k))
                nc.tensor.matmul(pst[t][0:cols, :], rr(t_ones[:, 0:cols], k),
                                 rr(wb[:], k), start=False, stop=True)

            # select + prelu (node-major [cols, odim])
            # masked terms then pairwise tree-add (depth 3, ops independent)
            terms = []
            for t in range(NDIR):
                if t < NDIR - 1 and t % 2 == 0:
                    continue  # odd t handles the (t-1, t) pair below
                if t == NDIR - 1:
                    tt = layerbuf.tile([128, odim], F32, tag=f"selp{t // 2}",
                                       name=f"selp{k}_{t}")
                    nc.vector.tensor_scalar(
                        out=tt[0:cols, :], in0=pst[t][0:cols, :],
                        scalar1=mcol[0:cols, t:t + 1], scalar2=None, op0=MULT)
                else:
                    t0 = t - 1
                    h = layerbuf.tile([128, odim], F32, tag="selh",
                                      name=f"selh{k}_{t}")
                    nc.vector.tensor_scalar(
                        out=h[0:cols, :], in0=pst[t0][0:cols, :],
                        scalar1=mcol[0:cols, t0:t0 + 1], scalar2=None, op0=MULT)
                    tt = layerbuf.tile([128, odim], F32, tag=f"selp{t // 2}",
                                       name=f"selp{k}_{t}")
                    nc.vector.scalar_tensor_tensor(
                        out=tt[0:cols, :], in0=pst[t][0:cols, :],
                        scalar=mcol[0:cols, t:t + 1], in1=h[0:cols, :],
                        op0=MULT, op1=ADD)
                terms.append(tt)
            rnd = 0
            while len(terms) > 1:
                nxt = []
                for i in range(0, len(terms) - 1, 2):
                    o = layerbuf.tile([128, odim], F32,
                                      tag=f"selr{rnd}_{i // 2}",
                                      name=f"selr{k}_{rnd}_{i // 2}")
                    nc.vector.tensor_add(o[0:cols, :], terms[i][0:cols, :],
                                         terms[i + 1][0:cols, :])
                    nxt.append(o)
                if len(terms) % 2:
                    nxt.append(terms[-1])
                terms = nxt
                rnd += 1
            xm_nm = layerbuf.tile([128, odim], F32, tag="xmnmT")
            prelu_store(terms[0][0:cols, :], xm_nm[0:cols, :], alphas[k])

            # transpose xm to feature-major chunks for the Ws contraction
            xfm = layerbuf.tile([128, odim // 128, BPC, n], F32, tag="xfmT")
            for f in range(odim // 128):
                pstr = pp.tile([128, 128], F32, tag="ps", name=f"pstr{k}_{f}")
                nc.tensor.transpose(pstr[:, 0:cols],
                                    xm_nm[0:cols, f * 128:(f + 1) * 128],
                                    t_ident[0:cols, 0:cols])
                nc.vector.tensor_copy(
                    rr(xfm[:, f, :, :].rearrange("p b n -> p (b n)"), k),
                    pstr[:, 0:cols])

            # Ws stage: stationary = [xfm chunks | sg chunks], streamed Ws
            psw = pp.tile([128, odim], F32, tag="ps", name=f"psw{k}")
            nsegs = len(c["segs"])
            for ci, (f, rows, src) in enumerate(c["segs"]):
                wt2 = wsp.tile([128, odim], F32, tag="wst")
                nc.sync.dma_start(out=rr(wt2[:], k), in_=rr(d[f"ws{k}"].ap()[ci], k))
                lhs = (xfm[0:rows, f, :, :] if src == "x"
                       else sg[k][0:rows, f, :, :]).rearrange("p b n -> p (b n)")
                nc.tensor.matmul(psw[0:cols, :], rr(lhs, k),
                                 rr(wt2[0:rows, :], k),
                                 start=(ci == 0), stop=False)
            # bias bs via ones-row (bs blob is [modim,1] = [odim,1] here)
            wb2 = wsp.tile([1, odim], F32, tag="wb2T", bufs=1)
            nc.sync.dma_start(out=rr(wb2[:], k),
                              in_=rr(d[f"bs{k}"].ap().rearrange("o i -> i o"), k))
            nc.tensor.matmul(psw[0:cols, :], rr(t_ones[:, 0:cols], k),
                             rr(wb2[:], k), start=False, stop=True)

            x_nm = layerbuf.tile([128, odim], F32, tag="xnmT")
            prelu_store(psw[0:cols, :], x_nm[0:cols, :], alphas[k])

            # transpose back to feature-major X[k]
            for f in range(odim // 128):
                pstr = pp.tile([128, 128], F32, tag="ps", name=f"pstrb{k}_{f}")
                nc.tensor.transpose(pstr[:, 0:cols],
                                    x_nm[0:cols, f * 128:(f + 1) * 128],
                                    t_ident[0:cols, 0:cols])
                nc.vector.tensor_copy(
                    X[k][:, f, :, :].rearrange("p b n -> p (b n)"),
                    pstr[:, 0:cols])
            kc = k + S
            if kc < NLAYERS:
                for f in range(odim // 128):
                    nc.vector.tensor_copy(
                        rr(sg[kc][:, f, :, :], kc), X[k][:, f, :, ::8])

        for f in range(4):
            nc.sync.dma_start(out=d["out"].ap()[f * 128:(f + 1) * 128, :],
                              in_=X[NLAYERS - 1][:, f, :, 0])
    return d


def kernel(**inputs):
    points = np.asarray(inputs["points"], np.float32)
    vecs = [np.asarray(inputs[f"vec_{k}"]) for k in range(1, NLAYERS)]
    in_maps, alphas, _ = _host_prep(points, inputs["dmap"], inputs["drev"],
                                    vecs, inputs["params"])
    nc = bacc.Bacc()
    _build(nc, alphas)
    nc.compile()
    res = run_bass_kernel_spmd(nc, in_maps, list(range(NCORES)))
    out = np.zeros((B, 512), np.float32)
    for core in range(NCORES):
        out[core * BPC:(core + 1) * BPC] = res.results[core]["out"].T
    return out


# revision 37
# speedup vs baseline: 1.0297x; 1.0297x over previous
"""Trainium2 Bass kernel for nn_Encoder (tree GNN message passing).

Data-parallel over batch: 8 cores x 4 batch items. Feature-major layout
(features on partitions, (batch, node) on the free dim). The per-node
direction select is folded into the matmul: activations are block-stacked
into Htilde (K = 7*2*idim rows + 7 bias rows) whose rows are ordered
[even-sourced region | odd-sourced region | one-hot bias rows], so each
128-row K-tile is built with 1-2 full-width fused DVE ops
  out = (vec_rep == tcol) * child
with tcol a per-partition scalar column. Weights are host-reordered to
match (dmap/drev baked in); the per-node bias bg[dmap[t]] rides as 7 extra
K-rows whose rhs is the one-hot of vec. Outputs use M-replication (weights
tiled along M) so layers with odim<128 still fill all 128 partitions —
which also makes X replicated with period odim, which in turn makes every
partition range of X a valid child operand. PReLU = Relu(z) - a*Relu(-z)
(2 ACT passes + 1 fused DVE op). fp32r matmuls for leaf/k<=7, exact fp32
for k>=8 (same speed at their column counts).
"""

import numpy as np
from contextlib import ExitStack

import concourse.bass as bass
import concourse.bacc as bacc
import concourse.tile as tile
from concourse import mybir
from concourse.bass_utils import run_bass_kernel_spmd

# ----- problem constants (hardcoded per harness contract) -----
B = 32
N = 8192
NLAYERS = 14
S = 3                      # SAMPLE_LAYERS
NDIR = 7
NCORES = 8
BPC = B // NCORES          # batches per core = 4

ODIMS = [8]
for _ in range(1, NLAYERS):
    ODIMS.append(min(ODIMS[-1] * 2, 512))

F32 = mybir.dt.float32
F32R = mybir.dt.float32r
U8 = mybir.dt.uint8
RELU = mybir.ActivationFunctionType.Relu
MULT = mybir.AluOpType.mult
ADD = mybir.AluOpType.add
ISEQ = mybir.AluOpType.is_equal

# per-layer matmul dtype: True -> fp32r (fast), False -> fp32 (exact)
USE_F32R = {k: True for k in range(NLAYERS)}
TAIL0 = 8                 # first node-major (weight-streaming) layer

NBLK = 16                        # front-end blocks (quarter-batches)
LEAF_COLS = BPC * N // NBLK      # 4096 leaf cols per block


def _cfg(k):
    idim, odim = ODIMS[k - 1], ODIMS[k]
    n = N >> k
    rep = max(1, 128 // odim)
    modim = odim * rep
    mtiles = modim // 128
    reg_o = ((NDIR * idim + 31) // 32) * 32   # odd region start (32-aligned)
    bias0 = reg_o + NDIR * idim               # bias rows start
    krows = bias0 + NDIR                      # total K rows
    nkt = (krows + 127) // 128
    c = dict(k=k, idim=idim, odim=odim, n=n, rep=rep, modim=modim,
             mtiles=mtiles, krows=krows, nkt=nkt, reg_o=reg_o, bias0=bias0)
    if k >= S:
        sdim = ODIMS[k - S]
        fk = max(1, odim // 128)
        fs = max(1, sdim // 128)
        # Ws chain segments: x-chunks then sg-chunks, each its own K-tile
        segs = [(f, min(128, odim - f * 128), "x") for f in range(fk)]
        segs += [(f, min(128, sdim - f * 128), "s") for f in range(fs)]
        c.update(sdim=sdim, fk=fk, fs=fs, segs=segs)
    return c


def _frags(k):
    """STT build fragments per K-tile: (kt, r0, span, region, sub).
    Row layout: e-region [0, 7*idim), pad, o-region [reg_o, reg_o+7*idim),
    bias rows [bias0, bias0+7). Within a region: t = off // idim,
    feature = off % idim, sub = feature // 128. All fragment starts are
    32-aligned (hardware partition-offset granularity)."""
    c = _cfg(k)
    idim = c["idim"]
    # regions extended to cover padding/bias rows (tcol=99 there -> writes 0,
    # later overwritten by the bias DMA where applicable)
    regions = [(0, 0, c["reg_o"]), (1, c["reg_o"], c["nkt"] * 128)]
    out = []
    for kt in range((c["bias0"] + 127) // 128):
        lo = kt * 128
        hi = min(lo + 128, ((c["krows"] + 31) // 32) * 32)
        for region, ra, rb in regions:
            a, bnd = max(lo, ra), min(hi, rb)
            if a >= bnd:
                continue
            off = a - ra
            sub = (off % idim) // 128
            out.append((kt, a - lo, bnd - a, region, sub))
    return out




def _host_prep(points, dmap, drev, vecs, params):
    dmap = np.asarray(dmap).astype(np.int64)
    drev = np.asarray(drev).astype(np.int64)
    pl = params["leaf"]
    leafW = np.tile(np.asarray(pl["W"], np.float32), (1, 16))        # [3,128]
    leafb = np.tile(np.asarray(pl["b"], np.float32), 16)[:, None]    # [128,1]
    alphas = {0: float(pl["a"])}

    wg_blobs, ws_blobs, bs_blobs, tcols = {}, {}, {}, {}
    wgs_blobs, bgm_blobs = {}, {}
    for k in range(1, NLAYERS):
        c = _cfg(k)
        p = params["layers"][k - 1]
        alphas[k] = float(p["a"])
        Wg = np.asarray(p["Wg"], np.float32)   # [7, 2idim, odim]
        bg = np.asarray(p["bg"], np.float32)   # [7, odim]
        idim = c["idim"]
        blob = np.zeros((c["nkt"] * 128, c["modim"]), np.float32)
        tc_ = np.full((c["nkt"] * 128,), 99.0, np.float32)
        for r in range(2):
            for t in range(NDIR):
                hf = int(drev[t]) ^ r
                g0 = r * c["reg_o"] + t * idim
                blob[g0:g0 + idim, :] = np.tile(
                    Wg[dmap[t]][hf * idim:(hf + 1) * idim], (1, c["rep"]))
                tc_[g0:g0 + idim] = t
        for t in range(NDIR):
            blob[c["bias0"] + t, :] = np.tile(bg[dmap[t]], c["rep"])
        wg_blobs[k] = blob.reshape(c["nkt"], 128, c["modim"])
        tcols[k] = tc_.reshape(c["nkt"], 128).T.copy()   # [128, nkt]
        if k >= TAIL0:
            # streaming layout, partition-major: [7, 128, nsub*odim] so each
            # direction's weights DMA as one transfer with 16KB/partition runs
            wgs_blobs[k] = np.ascontiguousarray(
                np.stack([Wg[dmap[t]] for t in range(NDIR)]).reshape(
                    NDIR, 2 * idim // 128, 128, c["odim"]).transpose(0, 2, 1, 3)
                .reshape(NDIR, 128, (2 * idim // 128) * c["odim"]))
            bgm_blobs[k] = np.stack([bg[dmap[t]] for t in range(NDIR)])  # [7, odim]
        if k >= S:
            Ws = np.tile(np.asarray(p["Ws"], np.float32), (1, c["rep"]))
            bs = np.asarray(p["bs"], np.float32)
            blob2 = np.zeros((len(c["segs"]), 128, c["modim"]), np.float32)
            r0 = 0
            for i, (f, rows, _src) in enumerate(c["segs"]):
                blob2[i, :rows, :] = Ws[r0:r0 + rows]
                r0 += rows
            ws_blobs[k] = blob2
            bs_blobs[k] = np.tile(bs, c["rep"])[:, None]   # [modim, 1]

    in_maps = []
    for core in range(NCORES):
        bsl = slice(core * BPC, (core + 1) * BPC)
        m = {
            "pts": np.ascontiguousarray(
                np.asarray(points[bsl], np.float32).reshape(BPC * N, 3).T),
            "leafW": leafW, "leafb": leafb, "leafnb": -leafb,
        }
        for k in range(1, NLAYERS):
            c = _cfg(k)
            v = np.asarray(vecs[k - 1][bsl], np.int64).reshape(1, -1)
            if k < TAIL0:
                m[f"vec{k}"] = v.astype(np.uint8)
                oh = np.zeros((NDIR, BPC * c["n"]), np.float32)
                oh[v[0], np.arange(BPC * c["n"])] = 1.0
                m[f"oh{k}"] = oh
                m[f"wg{k}"] = wg_blobs[k]
                m[f"tc{k}"] = tcols[k]
            else:
                m[f"rrep{k}"] = drev[v].astype(np.uint8)        # [1, cols]
                m[f"vcol{k}"] = v.astype(np.float32).T          # [cols, 1]
                m[f"wgs{k}"] = wgs_blobs[k]
                m[f"bgm{k}"] = bgm_blobs[k]
            if k >= S:
                m[f"ws{k}"] = ws_blobs[k]
                m[f"bs{k}"] = bs_blobs[k]
                m[f"nbs{k}"] = -bs_blobs[k]
        in_maps.append(m)
    return in_maps, alphas, drev


def _build(nc, alphas):
    def rr(ap, k):
        return ap.bitcast(F32R) if USE_F32R[k] else ap

    d = {}
    d["pts"] = nc.declare_dram_parameter("pts", [3, BPC * N], F32, isOutput=False)
    d["leafW"] = nc.declare_dram_parameter("leafW", [3, 128], F32, isOutput=False)
    d["leafb"] = nc.declare_dram_parameter("leafb", [128, 1], F32, isOutput=False)
    d["leafnb"] = nc.declare_dram_parameter("leafnb", [128, 1], F32, isOutput=False)
    for k in range(1, NLAYERS):
        c = _cfg(k)
        cols = BPC * c["n"]
        if k < TAIL0:
            d[f"vec{k}"] = nc.declare_dram_parameter(f"vec{k}", [1, cols], U8, isOutput=False)
            d[f"oh{k}"] = nc.declare_dram_parameter(f"oh{k}", [NDIR, cols], F32, isOutput=False)
            d[f"wg{k}"] = nc.declare_dram_parameter(
                f"wg{k}", [c["nkt"], 128, c["modim"]], F32, isOutput=False)
            d[f"tc{k}"] = nc.declare_dram_parameter(f"tc{k}", [128, c["nkt"]], F32, isOutput=False)
        else:
            d[f"rrep{k}"] = nc.declare_dram_parameter(f"rrep{k}", [1, cols], U8, isOutput=False)
            d[f"vcol{k}"] = nc.declare_dram_parameter(f"vcol{k}", [cols, 1], F32, isOutput=False)
            d[f"wgs{k}"] = nc.declare_dram_parameter(
                f"wgs{k}", [NDIR, 128, (2 * c["idim"] // 128) * c["odim"]], F32, isOutput=False)
            d[f"bgm{k}"] = nc.declare_dram_parameter(f"bgm{k}", [NDIR, c["odim"]], F32, isOutput=False)
        if k >= S:
            d[f"ws{k}"] = nc.declare_dram_parameter(
                f"ws{k}", [len(c["segs"]), 128, c["modim"]], F32, isOutput=False)
            d[f"bs{k}"] = nc.declare_dram_parameter(f"bs{k}", [c["modim"], 1], F32, isOutput=False)
            d[f"nbs{k}"] = nc.declare_dram_parameter(f"nbs{k}", [c["modim"], 1], F32, isOutput=False)
    d["out"] = nc.declare_dram_parameter("out", [512, BPC], F32, isOutput=True)

    with tile.TileContext(nc) as tc, ExitStack() as ctx:
        persist = ctx.enter_context(tc.tile_pool(name="persist", bufs=1))
        layerbuf = ctx.enter_context(tc.tile_pool(name="layerbuf", bufs=1))
        layer2 = ctx.enter_context(tc.tile_pool(name="layer2", bufs=2))
        htp = ctx.enter_context(tc.tile_pool(name="ht", bufs=2))
        wsp = ctx.enter_context(tc.tile_pool(name="ws", bufs=2))
        pp = ctx.enter_context(tc.tile_pool(name="ps", bufs=8, space="PSUM"))
        tp = ctx.enter_context(tc.tile_pool(name="tmp", bufs=2))

        t_leafW = persist.tile([3, 128], F32, tag="leafW")
        t_leafb = persist.tile([128, 1], F32, tag="leafb")
        t_leafnb = persist.tile([128, 1], F32, tag="leafnb")
        nc.sync.dma_start(out=rr(t_leafW[:], 0), in_=rr(d["leafW"].ap(), 0))
        nc.sync.dma_start(out=t_leafb, in_=d["leafb"].ap())
        nc.sync.dma_start(out=t_leafnb, in_=d["leafnb"].ap())
        t_bs, t_nbs, t_tc = {}, {}, {}
        for k in range(1, NLAYERS):
            c = _cfg(k)
            if k < TAIL0:
                t_tc[k] = persist.tile([128, c["nkt"]], F32, tag=f"tc{k}", name=f"tc{k}")
                nc.sync.dma_start(out=t_tc[k], in_=d[f"tc{k}"].ap())
            if k >= S:
                t_bs[k] = persist.tile([128, c["mtiles"]], F32, tag=f"bs{k}", name=f"bs{k}")
                t_nbs[k] = persist.tile([128, c["mtiles"]], F32, tag=f"nbs{k}", name=f"nbs{k}")
                nc.sync.dma_start(out=t_bs[k], in_=d[f"bs{k}"].ap().rearrange(
                    "(m p) o -> p (m o)", p=128))
                nc.sync.dma_start(out=t_nbs[k], in_=d[f"nbs{k}"].ap().rearrange(
                    "(m p) o -> p (m o)", p=128))

        from concourse.masks import make_identity
        t_ident = persist.tile([128, 128], F32, tag="ident")
        make_identity(nc, t_ident[:])
        t_ones = persist.tile([1, 128], F32, tag="ones")
        nc.vector.memset(t_ones[:], 1.0)

        X = {}
        for k in range(S, NLAYERS):
            c = _cfg(k)
            X[k] = persist.tile([128, max(1, c["odim"] // 128), BPC, c["n"]],
                                F32, tag=f"X{k}", name=f"X{k}")
        sg = {}
        for k in range(S + 1, NLAYERS):
            c = _cfg(k)
            sg[k] = persist.tile([128, c["fs"], BPC, c["n"]], F32, tag=f"sg{k}", name=f"sg{k}")

        def prelu_store(ps_ap, out_ap, alpha, bias=None, nbias=None, eng=None):
            # prelu(z+b) = pos + a*((z+b) - pos), pos = relu(z+b)
            # 1 ACT pass + 2 DVE ops
            shape = [ps_ap.partition_size(), ps_ap.free_size()]
            pos = tp.tile(shape, F32, tag="pos")
            neg = tp.tile(shape, F32, tag="neg")
            nc.scalar.activation(pos[:], ps_ap, RELU,
                                 bias=(bias if bias is not None else 0.0), scale=1.0)
            nc.vector.scalar_tensor_tensor(
                out=neg[:], in0=ps_ap,
                scalar=(bias if bias is not None else 0.0), in1=pos[:],
                op0=ADD, op1=mybir.AluOpType.subtract)
            nc.vector.scalar_tensor_tensor(
                out=out_ap, in0=neg[:], scalar=alpha, in1=pos[:],
                op0=MULT, op1=ADD)

        # ------------ front-end: leaf + k=1..3, pairs of blocks stage-major ------------
        # Front-end pools live only for this phase; their SBUF is returned
        # before the deep back-end weight-prefetch pool opens.
        fe_ctx = ExitStack()
        fe2 = fe_ctx.enter_context(tc.tile_pool(name="fe2", bufs=2))
        wp = fe_ctx.enter_context(tc.tile_pool(name="fwp", bufs=4))
        BLKB = NBLK // BPC               # blocks per batch item
        X0bs, xprevs = {}, {}

        def fe_leaf(blk):
            X0b = fe2.tile([128, LEAF_COLS], F32, tag="X0b", name=f"X0b{blk}")
            for cc in range(LEAF_COLS // 512):
                t_pts = fe2.tile([3, 512], F32, tag="pts", name=f"pts{blk}_{cc}")
                nc.sync.dma_start(
                    out=rr(t_pts[:], 0),
                    in_=rr(d["pts"].ap()[:, blk * LEAF_COLS + cc * 512:
                                         blk * LEAF_COLS + (cc + 1) * 512], 0))
                ps = pp.tile([128, 512], F32, tag="ps", name=f"lps{blk}_{cc}")
                nc.tensor.matmul(ps[:], rr(t_leafW[:], 0), rr(t_pts[:], 0),
                                 start=True, stop=True)
                prelu_store(ps[:], rr(X0b[:, cc * 512:(cc + 1) * 512], S),
                            alphas[0], bias=t_leafb[:], nbias=t_leafnb[:])
            X0bs[blk] = X0b
            xprevs[blk] = X0b

        def fe_layer(blk, k):
            b, half = blk // BLKB, blk % BLKB
            X0b = X0bs[blk]
            xb_prev = xprevs[blk]
            c = _cfg(k)
            idim = c["idim"]
            cols = LEAF_COLS >> k
            col0 = blk * cols
            vrep = fe2.tile([128, cols], U8, tag="vrep", name=f"vrep{blk}_{k}")
            vap = d[f"vec{k}"].ap()
            nc.sync.dma_start(out=vrep, in_=bass.AP(
                tensor=vap.tensor, offset=vap.offset + col0,
                ap=[[0, 128], [1, cols]]))
            ht = fe2.tile([128, c["nkt"], cols], F32, tag="fht", name=f"fht{blk}_{k}")
            for kt, r0, span, region, sub in _frags(k):
                nc.vector.scalar_tensor_tensor(
                    out=rr(ht[r0:r0 + span, kt, :], k),
                    in0=vrep[r0:r0 + span, :],
                    scalar=t_tc[k][r0:r0 + span, kt:kt + 1],
                    in1=xb_prev[r0:r0 + span, region::2],
                    op0=ISEQ, op1=MULT)
            bkt, br0 = divmod(c["bias0"], 128)
            nc.sync.dma_start(
                out=rr(ht[br0:br0 + NDIR, bkt, :], k),
                in_=rr(d[f"oh{k}"].ap()[:, col0:col0 + cols], k))
            xm = fe2.tile([128, cols], F32, tag=f"xm{k}", name=f"xm{blk}_{k}")
            nccs = max(1, cols // 512)
            ccw = min(512, cols)
            pss = [pp.tile([128, ccw], F32, tag="ps", name=f"fps{blk}_{k}_{i}")
                   for i in range(nccs)]
            for kt in range(c["nkt"]):
                rows = min(128, c["krows"] - kt * 128)
                wt = wp.tile([128, c["modim"]], F32, tag="wt")
                nc.sync.dma_start(out=rr(wt[:], k), in_=rr(d[f"wg{k}"].ap()[kt], k))
                for cc in range(nccs):
                    nc.tensor.matmul(
                        pss[cc][:], rr(wt[0:rows, 0:128], k),
                        rr(ht[0:rows, kt, cc * ccw:(cc + 1) * ccw], k),
                        start=(kt == 0), stop=(kt == c["nkt"] - 1))
            if k < S:
                for cc in range(nccs):
                    prelu_store(pss[cc][:], xm[:, cc * ccw:(cc + 1) * ccw],
                                alphas[k])
                kc = k + S
                scols = cols // 8
                nc.vector.tensor_copy(
                    rr(sg[kc][:, 0, b, half * scols:(half + 1) * scols], kc),
                    xm[:, ::8])
                xprevs[blk] = xm
            else:
                for cc in range(nccs):
                    prelu_store(pss[cc][:], rr(xm[:, cc * ccw:(cc + 1) * ccw], k),
                                alphas[k])
                ps2 = pp.tile([128, cols], F32, tag="ps", name=f"wps{blk}")
                for ci, (f, rows, src) in enumerate(c["segs"]):
                    wt2 = wsp.tile([128, c["modim"]], F32, tag="wst")
                    nc.sync.dma_start(out=rr(wt2[:], k),
                                      in_=rr(d[f"ws{k}"].ap()[ci], k))
                    rhs = (xm[0:rows, :] if src == "x"
                           else X0b[0:rows, ::8])
                    nc.tensor.matmul(ps2[:], rr(wt2[0:rows, 0:128], k),
                                     rr(rhs, k), start=(ci == 0),
                                     stop=(ci == len(c["segs"]) - 1))
                prelu_store(ps2[:],
                            X[S][:, 0, b, half * cols:(half + 1) * cols],
                            alphas[k], bias=t_bs[k][:, 0:1], nbias=t_nbs[k][:, 0:1])

        for pb in range(0, NBLK, 2):
            pair = (pb, pb + 1)
            for blk in pair:
                fe_leaf(blk)
            for k in range(1, S + 1):
                for blk in pair:
                    fe_layer(blk, k)
        fe_ctx.close()
        wp = ctx.enter_context(tc.tile_pool(name="w", bufs=24))

        # sg for layer 6 comes from the completed X[3]
        nc.vector.tensor_copy(rr(sg[6][:, 0, :, :], 6), X[S][:, 0, :, ::8])

        # ---------------- back-end (feature-major): k = 4..TAIL0-1 ----------------
        for k in range(S + 1, TAIL0):
            c = _cfg(k)
            n, idim, odim = c["n"], c["idim"], c["odim"]
            cols = BPC * n
            vrep = layer2.tile([128, BPC, n], U8, tag="vrepB")
            vap = d[f"vec{k}"].ap()
            nc.sync.dma_start(out=vrep, in_=bass.AP(
                tensor=vap.tensor, offset=vap.offset,
                ap=[[0, 128], [n, BPC], [1, n]]))
            t_oh = layerbuf.tile([NDIR, BPC, n], F32, tag="ohB")
            nc.sync.dma_start(
                out=rr(t_oh[:], k),
                in_=rr(d[f"oh{k}"].ap().rearrange("p (b n) -> p b n", b=BPC), k))

            nccs = max(1, cols // 512)
            ccw = min(512, cols)
            pss = {(m, cc): pp.tile([128, ccw], F32, tag="ps", name=f"ps{k}_{m}_{cc}")
                   for m in range(c["mtiles"]) for cc in range(nccs)}
            xprev = X[k - 1]
            frags = _frags(k)
            nht = c["nkt"] - 1           # Htilde K-tiles (last tile is bias-only)
            gkt = max(1, 1024 // cols)   # K-tiles per ht group (~4KB)
            for kt0 in range(0, nht, gkt):
                kts = range(kt0, min(kt0 + gkt, nht))
                ht = htp.tile([128, len(kts), BPC, n], F32, tag="bht")
                for kt, r0, span, region, sub in frags:
                    if kt not in kts:
                        continue
                    nc.vector.scalar_tensor_tensor(
                        out=rr(ht[r0:r0 + span, kt - kt0, :, :], k),
                        in0=vrep[r0:r0 + span, :, :],
                        scalar=t_tc[k][r0:r0 + span, kt:kt + 1],
                        in1=xprev[r0:r0 + span, sub, :, region::2],
                        op0=ISEQ, op1=MULT)
                for kt in kts:
                    wt = wp.tile([128, c["modim"]], F32, tag="wt")
                    nc.sync.dma_start(out=rr(wt[:], k), in_=rr(d[f"wg{k}"].ap()[kt], k))
                    rhs2d = ht[:, kt - kt0, :, :].rearrange("p b n -> p (b n)")
                    for m in range(c["mtiles"]):
                        for cc in range(nccs):
                            nc.tensor.matmul(
                                pss[(m, cc)][:],
                                rr(wt[:, m * 128:(m + 1) * 128], k),
                                rr(rhs2d[:, cc * ccw:(cc + 1) * ccw], k),
                                start=(kt == 0), stop=False)
            wt = wp.tile([128, c["modim"]], F32, tag="wt")
            nc.sync.dma_start(out=rr(wt[:], k), in_=rr(d[f"wg{k}"].ap()[c["nkt"] - 1], k))
            oh2d = t_oh[:].rearrange("p b n -> p (b n)")
            for m in range(c["mtiles"]):
                for cc in range(nccs):
                    nc.tensor.matmul(
                        pss[(m, cc)][:], rr(wt[0:NDIR, m * 128:(m + 1) * 128], k),
                        rr(oh2d[:, cc * ccw:(cc + 1) * ccw], k),
                        start=False, stop=True)
            xm = layerbuf.tile([128, c["mtiles"], BPC, n], F32, tag="xmB")
            for m in range(c["mtiles"]):
                xm2d = xm[:, m, :, :].rearrange("p b n -> p (b n)")
                for cc in range(nccs):
                    prelu_store(pss[(m, cc)][:],
                                rr(xm2d[:, cc * ccw:(cc + 1) * ccw], k), alphas[k])
            pss2 = {(m, cc): pp.tile([128, ccw], F32, tag="ps", name=f"ps2_{k}_{m}_{cc}")
                    for m in range(c["mtiles"]) for cc in range(nccs)}
            for ci, (f, rows, src) in enumerate(c["segs"]):
                wt2 = wsp.tile([128, c["modim"]], F32, tag="wst")
                nc.sync.dma_start(out=rr(wt2[:], k), in_=rr(d[f"ws{k}"].ap()[ci], k))
                rhs2d = (xm[0:rows, f, :, :] if src == "x"
                         else sg[k][0:rows, f, :, :]).rearrange("p b n -> p (b n)")
                for m in range(c["mtiles"]):
                    for cc in range(nccs):
                        nc.tensor.matmul(
                            pss2[(m, cc)][:],
                            rr(wt2[0:rows, m * 128:(m + 1) * 128], k),
                            rr(rhs2d[:, cc * ccw:(cc + 1) * ccw], k),
                            start=(ci == 0), stop=(ci == len(c["segs"]) - 1))
            for m in range(c["mtiles"]):
                x2d = X[k][:, m, :, :].rearrange("p b n -> p (b n)")
                for cc in range(nccs):
                    prelu_store(pss2[(m, cc)][:], x2d[:, cc * ccw:(cc + 1) * ccw],
                                alphas[k], bias=t_bs[k][:, m:m + 1],
                                nbias=t_nbs[k][:, m:m + 1])
            kc = k + S
            if kc < NLAYERS:
                for f in range(max(1, odim // 128)):
                    nc.vector.tensor_copy(
                        rr(sg[kc][:, f, :, :], kc), X[k][:, f, :, ::8])

        # ---------------- tail (node-major, weight-streaming): k >= TAIL0 ----------------
        for k in range(TAIL0, NLAYERS):
            c = _cfg(k)
            n, idim, odim = c["n"], c["idim"], c["odim"]
            cols = BPC * n
            nsub = 2 * idim // 128          # ht K-tiles (8)
            xprev = X[k - 1]
            fprev = ODIMS[k - 1] // 128     # 4

            rrep = layer2.tile([128, BPC, n], U8, tag="rrepT", bufs=1)
            rap = d[f"rrep{k}"].ap()
            nc.sync.dma_start(out=rrep, in_=bass.AP(
                tensor=rap.tensor, offset=rap.offset,
                ap=[[0, 128], [n, BPC], [1, n]]))
            vcol = layer2.tile([128, 1], F32, tag="vcolT")
            nc.sync.dma_start(out=vcol[0:cols, :], in_=d[f"vcol{k}"].ap())
            mcol = layer2.tile([128, NDIR], F32, tag="mcolT")
            for t in range(NDIR):
                nc.vector.tensor_scalar(
                    out=mcol[0:cols, t:t + 1], in0=vcol[0:cols, :],
                    scalar1=float(t), scalar2=None, op0=ISEQ)

            # ht: [128, nsub, BPC, n]; rows of sub j = features of chunk j%4,
            # half j//4. top half: r=0 -> even child; bottom: r=0 -> odd.
            ht = htp.tile([128, nsub, BPC, n], F32, tag="tht")
            rrep4 = rrep[:].unsqueeze(1).broadcast_to([128, fprev, BPC, n])
            for half in range(2):
                ev = xprev[:, :, :, 0::2]
                od = xprev[:, :, :, 1::2]
                a_src = ev if half == 0 else od   # used when r == 0
                b_src = od if half == 0 else ev   # used when r == 1
                tmp1 = tp.tile([128, fprev, BPC, n], F32, tag="sel1", bufs=1)
                tmp2 = tp.tile([128, fprev, BPC, n], F32, tag="sel2", bufs=1)
                nc.vector.scalar_tensor_tensor(
                    out=tmp1[:], in0=rrep4, scalar=0.0, in1=a_src,
                    op0=ISEQ, op1=MULT)
                nc.vector.scalar_tensor_tensor(
                    out=tmp2[:], in0=rrep4, scalar=1.0, in1=b_src,
                    op0=ISEQ, op1=MULT)
                nc.vector.tensor_add(
                    rr(ht[:, half * fprev:(half + 1) * fprev, :, :], k),
                    tmp1[:], tmp2[:])

            # 7 direction matmuls: stationary ht tiles, streamed weights
            pst = {}
            for t in range(NDIR):
                pst[t] = pp.tile([128, odim], F32, tag="ps", name=f"pst{k}_{t}")
                for j in range(nsub):
                    wt = wp.tile([128, odim], F32, tag="wt")
                    nc.sync.dma_start(
                        out=rr(wt[:], k),
                        in_=rr(d[f"wgs{k}"].ap()[t][:, j * odim:(j + 1) * odim], k))
                    nc.tensor.matmul(
                        pst[t][0:cols, :],
                        rr(ht[:, j, :, :].rearrange("p b n -> p (b n)"), k),
                        rr(wt[:], k), start=(j == 0), stop=False)
                # bias: ones-row x bg[dmap[t]]
                wb = wsp.tile([1, odim], F32, tag="wbT", bufs=1)
                nc.sync.dma_start(out=rr(wb[:], k), in_=rr(d[f"bgm{k}"].ap()[t:t + 1, :], k))
                nc.tensor.matmul(pst[t][0:cols, :], rr(t_ones[:, 0:cols], k),
                                 rr(wb[:], k), start=False, stop=True)

            # select + prelu (node-major [cols, odim])
            # masked terms then pairwise tree-add (depth 3, ops independent)
            terms = []
            for t in range(NDIR):
                if t < NDIR - 1 and t % 2 == 0:
                    continue  # odd t handles the (t-1, t) pair below
                if t == NDIR - 1:
                    tt = layerbuf.tile([128, odim], F32, tag=f"selp{t // 2}",
                                       name=f"selp{k}_{t}")
                    nc.vector.tensor_scalar(
                        out=tt[0:cols, :], in0=pst[t][0:cols, :],
                        scalar1=mcol[0:cols, t:t + 1], scalar2=None, op0=MULT)
                else:
                    t0 = t - 1
                    h = layerbuf.tile([128, odim], F32, tag="selh",
                                      name=f"selh{k}_{t}")
                    nc.vector.tensor_scalar(
                        out=h[0:cols, :], in0=pst[t0][0:cols, :],
                        scalar1=mcol[0:cols, t0:t0 + 1], scalar2=None, op0=MULT)
                    tt = layerbuf.tile([128, odim], F32, tag=f"selp{t // 2}",
                                       name=f"selp{k}_{t}")
                    nc.vector.scalar_tensor_tensor(
                        out=tt[0:cols, :], in0=pst[t][0:cols, :],
                        scalar=mcol[0:cols, t:t + 1], in1=h[0:cols, :],
                        op0=MULT, op1=ADD)
                terms.append(tt)
            rnd = 0
            while len(terms) > 1:
                nxt = []
                for i in range(0, len(terms) - 1, 2):
                    o = layerbuf.tile([128, odim], F32,
                                      tag=f"selr{rnd}_{i // 2}",
                                      name=f"selr{k}_{rnd}_{i // 2}")
                    nc.vector.tensor_add(o[0:cols, :], terms[i][0:cols, :],
                                         terms[i + 1][0:cols, :])
                    nxt.append(o)
                if len(terms) % 2:
                    nxt.append(terms[-1])
                terms = nxt
                rnd += 1
            xm_nm = layerbuf.tile([128, odim], F32, tag="xmnmT")
            prelu_store(terms[0][0:cols, :], xm_nm[0:cols, :], alphas[k])

            # transpose xm to feature-major chunks for the Ws contraction
            xfm = layerbuf.tile([128, odim // 128, BPC, n], F32, tag="xfmT")
            for f in range(odim // 128):
                pstr = pp.tile([128, 128], F32, tag="ps", name=f"pstr{k}_{f}")
                nc.tensor.transpose(pstr[:, 0:cols],
                                    xm_nm[0:cols, f * 128:(f + 1) * 128],
                                    t_ident[0:cols, 0:cols])
                nc.vector.tensor_copy(
                    rr(xfm[:, f, :, :].rearrange("p b n -> p (b n)"), k),
                    pstr[:, 0:cols])

            # Ws stage: stationary = [xfm chunks | sg chunks], streamed Ws
            psw = pp.tile([128, odim], F32, tag="ps", name=f"psw{k}")
            nsegs = len(c["segs"])
            for ci, (f, rows, src) in enumerate(c["segs"]):
                wt2 = wsp.tile([128, odim], F32, tag="wst")
                nc.sync.dma_start(out=rr(wt2[:], k), in_=rr(d[f"ws{k}"].ap()[ci], k))
                lhs = (xfm[0:rows, f, :, :] if src == "x"
                       else sg[k][0:rows, f, :, :]).rearrange("p b n -> p (b n)")
                nc.tensor.matmul(psw[0:cols, :], rr(lhs, k),
                                 rr(wt2[0:rows, :], k),
                                 start=(ci == 0), stop=False)
            # bias bs via ones-row (bs blob is [modim,1] = [odim,1] here)
            wb2 = wsp.tile([1, odim], F32, tag="wb2T", bufs=1)
            nc.sync.dma_start(out=rr(wb2[:], k),
                              in_=rr(d[f"bs{k}"].ap().rearrange("o i -> i o"), k))
            nc.tensor.matmul(psw[0:cols, :], rr(t_ones[:, 0:cols], k),
                             rr(wb2[:], k), start=False, stop=True)

            x_nm = layerbuf.tile([128, odim], F32, tag="xnmT")
            prelu_store(psw[0:cols, :], x_nm[0:cols, :], alphas[k])

            # transpose back to feature-major X[k]
            for f in range(odim // 128):
                pstr = pp.tile([128, 128], F32, tag="ps", name=f"pstrb{k}_{f}")
                nc.tensor.transpose(pstr[:, 0:cols],
                                    x_nm[0:cols, f * 128:(f + 1) * 128],
                                    t_ident[0:cols, 0:cols])
                nc.vector.tensor_copy(
                    X[k][:, f, :, :].rearrange("p b n -> p (b n)"),
                    pstr[:, 0:cols])
            kc = k + S
            if kc < NLAYERS:
                for f in range(odim // 128):
                    nc.vector.tensor_copy(
                        rr(sg[kc][:, f, :, :], kc), X[k][:, f, :, ::8])

        for f in range(4):
            nc.sync.dma_start(out=d["out"].ap()[f * 128:(f + 1) * 128, :],
                              in_=X[NLAYERS - 1][:, f, :, 0])
    return d


def kernel(**inputs):
    points = np.asarray(inputs["points"], np.float32)
    vecs = [np.asarray(inputs[f"vec_{k}"]) for k in range(1, NLAYERS)]
    in_maps, alphas, _ = _host_prep(points, inputs["dmap"], inputs["drev"],
                                    vecs, inputs["params"])
    nc = bacc.Bacc()
    _build(nc, alphas)
    nc.compile()
    res = run_bass_kernel_spmd(nc, in_maps, list(range(NCORES)))
    out = np.zeros((B, 512), np.float32)
    for core in range(NCORES):
        out[core * BPC:(core + 1) * BPC] = res.results[core]["out"].T
    return out


# revision 38
# speedup vs baseline: 1.0830x; 1.0517x over previous
"""Trainium2 Bass kernel for nn_Encoder (tree GNN message passing).

Data-parallel over batch: 8 cores x 4 batch items. Feature-major layout
(features on partitions, (batch, node) on the free dim). The per-node
direction select is folded into the matmul: activations are block-stacked
into Htilde (K = 7*2*idim rows + 7 bias rows) whose rows are ordered
[even-sourced region | odd-sourced region | one-hot bias rows], so each
128-row K-tile is built with 1-2 full-width fused DVE ops
  out = (vec_rep == tcol) * child
with tcol a per-partition scalar column. Weights are host-reordered to
match (dmap/drev baked in); the per-node bias bg[dmap[t]] rides as 7 extra
K-rows whose rhs is the one-hot of vec. Outputs use M-replication (weights
tiled along M) so layers with odim<128 still fill all 128 partitions —
which also makes X replicated with period odim, which in turn makes every
partition range of X a valid child operand. PReLU = Relu(z) - a*Relu(-z)
(2 ACT passes + 1 fused DVE op). fp32r matmuls for leaf/k<=7, exact fp32
for k>=8 (same speed at their column counts).
"""

import numpy as np
from contextlib import ExitStack

import concourse.bass as bass
import concourse.bacc as bacc
import concourse.tile as tile
from concourse import mybir
from concourse.bass_utils import run_bass_kernel_spmd

# ----- problem constants (hardcoded per harness contract) -----
B = 32
N = 8192
NLAYERS = 14
S = 3                      # SAMPLE_LAYERS
NDIR = 7
NCORES = 8
BPC = B // NCORES          # batches per core = 4

ODIMS = [8]
for _ in range(1, NLAYERS):
    ODIMS.append(min(ODIMS[-1] * 2, 512))

F32 = mybir.dt.float32
F32R = mybir.dt.float32r
U8 = mybir.dt.uint8
RELU = mybir.ActivationFunctionType.Relu
MULT = mybir.AluOpType.mult
ADD = mybir.AluOpType.add
ISEQ = mybir.AluOpType.is_equal

# per-layer matmul dtype: True -> fp32r (fast), False -> fp32 (exact)
USE_F32R = {k: True for k in range(NLAYERS)}
TAIL0 = 8                 # first node-major (weight-streaming) layer

NBLK = 8                         # front-end blocks (half-batches)
LEAF_COLS = BPC * N // NBLK      # 4096 leaf cols per block


def _cfg(k):
    idim, odim = ODIMS[k - 1], ODIMS[k]
    n = N >> k
    rep = max(1, 128 // odim)
    modim = odim * rep
    mtiles = modim // 128
    reg_o = ((NDIR * idim + 31) // 32) * 32   # odd region start (32-aligned)
    bias0 = reg_o + NDIR * idim               # bias rows start
    krows = bias0 + NDIR                      # total K rows
    nkt = (krows + 127) // 128
    c = dict(k=k, idim=idim, odim=odim, n=n, rep=rep, modim=modim,
             mtiles=mtiles, krows=krows, nkt=nkt, reg_o=reg_o, bias0=bias0)
    if k >= S:
        sdim = ODIMS[k - S]
        fk = max(1, odim // 128)
        fs = max(1, sdim // 128)
        # Ws chain segments: x-chunks then sg-chunks, each its own K-tile
        segs = [(f, min(128, odim - f * 128), "x") for f in range(fk)]
        segs += [(f, min(128, sdim - f * 128), "s") for f in range(fs)]
        c.update(sdim=sdim, fk=fk, fs=fs, segs=segs)
    return c


def _frags(k):
    """STT build fragments per K-tile: (kt, r0, span, region, sub).
    Row layout: e-region [0, 7*idim), pad, o-region [reg_o, reg_o+7*idim),
    bias rows [bias0, bias0+7). Within a region: t = off // idim,
    feature = off % idim, sub = feature // 128. All fragment starts are
    32-aligned (hardware partition-offset granularity)."""
    c = _cfg(k)
    idim = c["idim"]
    # regions extended to cover padding/bias rows (tcol=99 there -> writes 0,
    # later overwritten by the bias DMA where applicable)
    regions = [(0, 0, c["reg_o"]), (1, c["reg_o"], c["nkt"] * 128)]
    out = []
    for kt in range((c["bias0"] + 127) // 128):
        lo = kt * 128
        hi = min(lo + 128, ((c["krows"] + 31) // 32) * 32)
        for region, ra, rb in regions:
            a, bnd = max(lo, ra), min(hi, rb)
            if a >= bnd:
                continue
            off = a - ra
            sub = (off % idim) // 128
            out.append((kt, a - lo, bnd - a, region, sub))
    return out




def _host_prep(points, dmap, drev, vecs, params):
    dmap = np.asarray(dmap).astype(np.int64)
    drev = np.asarray(drev).astype(np.int64)
    pl = params["leaf"]
    leafW = np.tile(np.asarray(pl["W"], np.float32), (1, 16))        # [3,128]
    leafb = np.tile(np.asarray(pl["b"], np.float32), 16)[:, None]    # [128,1]
    alphas = {0: float(pl["a"])}

    wg_blobs, ws_blobs, bs_blobs, tcols = {}, {}, {}, {}
    wgs_blobs, bgm_blobs = {}, {}
    for k in range(1, NLAYERS):
        c = _cfg(k)
        p = params["layers"][k - 1]
        alphas[k] = float(p["a"])
        Wg = np.asarray(p["Wg"], np.float32)   # [7, 2idim, odim]
        bg = np.asarray(p["bg"], np.float32)   # [7, odim]
        idim = c["idim"]
        blob = np.zeros((c["nkt"] * 128, c["modim"]), np.float32)
        tc_ = np.full((c["nkt"] * 128,), 99.0, np.float32)
        for r in range(2):
            for t in range(NDIR):
                hf = int(drev[t]) ^ r
                g0 = r * c["reg_o"] + t * idim
                blob[g0:g0 + idim, :] = np.tile(
                    Wg[dmap[t]][hf * idim:(hf + 1) * idim], (1, c["rep"]))
                tc_[g0:g0 + idim] = t
        for t in range(NDIR):
            blob[c["bias0"] + t, :] = np.tile(bg[dmap[t]], c["rep"])
        wg_blobs[k] = blob.reshape(c["nkt"], 128, c["modim"])
        tcols[k] = tc_.reshape(c["nkt"], 128).T.copy()   # [128, nkt]
        if k >= TAIL0:
            # streaming layout, partition-major: [7, 128, nsub*odim] so each
            # direction's weights DMA as one transfer with 16KB/partition runs
            wgs_blobs[k] = np.ascontiguousarray(
                np.stack([Wg[dmap[t]] for t in range(NDIR)]).reshape(
                    NDIR, 2 * idim // 128, 128, c["odim"]).transpose(0, 2, 1, 3)
                .reshape(NDIR, 128, (2 * idim // 128) * c["odim"]))
            bgm_blobs[k] = np.stack([bg[dmap[t]] for t in range(NDIR)])  # [7, odim]
        if k >= S:
            Ws = np.tile(np.asarray(p["Ws"], np.float32), (1, c["rep"]))
            bs = np.asarray(p["bs"], np.float32)
            blob2 = np.zeros((len(c["segs"]), 128, c["modim"]), np.float32)
            r0 = 0
            for i, (f, rows, _src) in enumerate(c["segs"]):
                blob2[i, :rows, :] = Ws[r0:r0 + rows]
                r0 += rows
            ws_blobs[k] = blob2
            bs_blobs[k] = np.tile(bs, c["rep"])[:, None]   # [modim, 1]

    in_maps = []
    for core in range(NCORES):
        bsl = slice(core * BPC, (core + 1) * BPC)
        m = {
            "pts": np.ascontiguousarray(
                np.asarray(points[bsl], np.float32).reshape(BPC * N, 3).T),
            "leafW": leafW, "leafb": leafb, "leafnb": -leafb,
        }
        for k in range(1, NLAYERS):
            c = _cfg(k)
            v = np.asarray(vecs[k - 1][bsl], np.int64).reshape(1, -1)
            if k < TAIL0:
                m[f"vec{k}"] = v.astype(np.uint8)
                oh = np.zeros((NDIR, BPC * c["n"]), np.float32)
                oh[v[0], np.arange(BPC * c["n"])] = 1.0
                m[f"oh{k}"] = oh
                m[f"wg{k}"] = wg_blobs[k]
                m[f"tc{k}"] = tcols[k]
            else:
                m[f"rrep{k}"] = drev[v].astype(np.uint8)        # [1, cols]
                m[f"vcol{k}"] = v.astype(np.float32).T          # [cols, 1]
                m[f"wgs{k}"] = wgs_blobs[k]
                m[f"bgm{k}"] = bgm_blobs[k]
            if k >= S:
                m[f"ws{k}"] = ws_blobs[k]
                m[f"bs{k}"] = bs_blobs[k]
                m[f"nbs{k}"] = -bs_blobs[k]
        in_maps.append(m)
    return in_maps, alphas, drev


def _build(nc, alphas):
    def rr(ap, k):
        return ap.bitcast(F32R) if USE_F32R[k] else ap

    d = {}
    d["pts"] = nc.declare_dram_parameter("pts", [3, BPC * N], F32, isOutput=False)
    d["leafW"] = nc.declare_dram_parameter("leafW", [3, 128], F32, isOutput=False)
    d["leafb"] = nc.declare_dram_parameter("leafb", [128, 1], F32, isOutput=False)
    d["leafnb"] = nc.declare_dram_parameter("leafnb", [128, 1], F32, isOutput=False)
    for k in range(1, NLAYERS):
        c = _cfg(k)
        cols = BPC * c["n"]
        if k < TAIL0:
            d[f"vec{k}"] = nc.declare_dram_parameter(f"vec{k}", [1, cols], U8, isOutput=False)
            d[f"oh{k}"] = nc.declare_dram_parameter(f"oh{k}", [NDIR, cols], F32, isOutput=False)
            d[f"wg{k}"] = nc.declare_dram_parameter(
                f"wg{k}", [c["nkt"], 128, c["modim"]], F32, isOutput=False)
            d[f"tc{k}"] = nc.declare_dram_parameter(f"tc{k}", [128, c["nkt"]], F32, isOutput=False)
        else:
            d[f"rrep{k}"] = nc.declare_dram_parameter(f"rrep{k}", [1, cols], U8, isOutput=False)
            d[f"vcol{k}"] = nc.declare_dram_parameter(f"vcol{k}", [cols, 1], F32, isOutput=False)
            d[f"wgs{k}"] = nc.declare_dram_parameter(
                f"wgs{k}", [NDIR, 128, (2 * c["idim"] // 128) * c["odim"]], F32, isOutput=False)
            d[f"bgm{k}"] = nc.declare_dram_parameter(f"bgm{k}", [NDIR, c["odim"]], F32, isOutput=False)
        if k >= S:
            d[f"ws{k}"] = nc.declare_dram_parameter(
                f"ws{k}", [len(c["segs"]), 128, c["modim"]], F32, isOutput=False)
            d[f"bs{k}"] = nc.declare_dram_parameter(f"bs{k}", [c["modim"], 1], F32, isOutput=False)
            d[f"nbs{k}"] = nc.declare_dram_parameter(f"nbs{k}", [c["modim"], 1], F32, isOutput=False)
    d["out"] = nc.declare_dram_parameter("out", [512, BPC], F32, isOutput=True)

    with tile.TileContext(nc) as tc, ExitStack() as ctx:
        persist = ctx.enter_context(tc.tile_pool(name="persist", bufs=1))
        wsp = ctx.enter_context(tc.tile_pool(name="ws", bufs=2))
        pp = ctx.enter_context(tc.tile_pool(name="ps", bufs=8, space="PSUM"))
        tp = ctx.enter_context(tc.tile_pool(name="tmp", bufs=2))

        t_leafW = persist.tile([3, 128], F32, tag="leafW")
        t_leafb = persist.tile([128, 1], F32, tag="leafb")
        t_leafnb = persist.tile([128, 1], F32, tag="leafnb")
        nc.sync.dma_start(out=rr(t_leafW[:], 0), in_=rr(d["leafW"].ap(), 0))
        nc.sync.dma_start(out=t_leafb, in_=d["leafb"].ap())
        nc.sync.dma_start(out=t_leafnb, in_=d["leafnb"].ap())
        t_bs, t_nbs, t_tc = {}, {}, {}
        for k in range(1, NLAYERS):
            c = _cfg(k)
            if k < TAIL0:
                t_tc[k] = persist.tile([128, c["nkt"]], F32, tag=f"tc{k}", name=f"tc{k}")
                nc.sync.dma_start(out=t_tc[k], in_=d[f"tc{k}"].ap())
            if k >= S:
                t_bs[k] = persist.tile([128, c["mtiles"]], F32, tag=f"bs{k}", name=f"bs{k}")
                t_nbs[k] = persist.tile([128, c["mtiles"]], F32, tag=f"nbs{k}", name=f"nbs{k}")
                nc.sync.dma_start(out=t_bs[k], in_=d[f"bs{k}"].ap().rearrange(
                    "(m p) o -> p (m o)", p=128))
                nc.sync.dma_start(out=t_nbs[k], in_=d[f"nbs{k}"].ap().rearrange(
                    "(m p) o -> p (m o)", p=128))

        from concourse.masks import make_identity
        t_ident = persist.tile([128, 128], F32, tag="ident")
        make_identity(nc, t_ident[:])
        t_ones = persist.tile([1, 128], F32, tag="ones")
        nc.vector.memset(t_ones[:], 1.0)

        X = {}
        for k in range(S, NLAYERS):
            c = _cfg(k)
            X[k] = persist.tile([128, max(1, c["odim"] // 128), BPC, c["n"]],
                                F32, tag=f"X{k}", name=f"X{k}")
        sg = {}
        for k in range(S + 1, NLAYERS):
            c = _cfg(k)
            sg[k] = persist.tile([128, c["fs"], BPC, c["n"]], F32, tag=f"sg{k}", name=f"sg{k}")

        def prelu_store(ps_ap, out_ap, alpha, bias=None, nbias=None, eng=None):
            # prelu(z+b) = pos + a*((z+b) - pos), pos = relu(z+b)
            # 1 ACT pass + 2 DVE ops
            shape = [ps_ap.partition_size(), ps_ap.free_size()]
            pos = tp.tile(shape, F32, tag="pos")
            neg = tp.tile(shape, F32, tag="neg")
            nc.scalar.activation(pos[:], ps_ap, RELU,
                                 bias=(bias if bias is not None else 0.0), scale=1.0)
            nc.vector.scalar_tensor_tensor(
                out=neg[:], in0=ps_ap,
                scalar=(bias if bias is not None else 0.0), in1=pos[:],
                op0=ADD, op1=mybir.AluOpType.subtract)
            nc.vector.scalar_tensor_tensor(
                out=out_ap, in0=neg[:], scalar=alpha, in1=pos[:],
                op0=MULT, op1=ADD)

        # ------------ front-end: leaf + k=1..3, pairs of blocks stage-major ------------
        # Front-end pools live only for this phase; their SBUF is returned
        # before the deep back-end weight-prefetch pool opens.
        fe_ctx = ExitStack()
        fe2 = fe_ctx.enter_context(tc.tile_pool(name="fe2", bufs=2))
        wp = fe_ctx.enter_context(tc.tile_pool(name="fwp", bufs=4))
        BLKB = NBLK // BPC               # blocks per batch item
        X0bs, xprevs = {}, {}

        def fe_leaf(blk):
            X0b = fe2.tile([128, LEAF_COLS], F32, tag="X0b", name=f"X0b{blk}")
            for cc in range(LEAF_COLS // 512):
                t_pts = fe2.tile([3, 512], F32, tag="pts", name=f"pts{blk}_{cc}")
                nc.sync.dma_start(
                    out=rr(t_pts[:], 0),
                    in_=rr(d["pts"].ap()[:, blk * LEAF_COLS + cc * 512:
                                         blk * LEAF_COLS + (cc + 1) * 512], 0))
                ps = pp.tile([128, 512], F32, tag="ps", name=f"lps{blk}_{cc}")
                nc.tensor.matmul(ps[:], rr(t_leafW[:], 0), rr(t_pts[:], 0),
                                 start=True, stop=True)
                prelu_store(ps[:], rr(X0b[:, cc * 512:(cc + 1) * 512], S),
                            alphas[0], bias=t_leafb[:], nbias=t_leafnb[:])
            X0bs[blk] = X0b
            xprevs[blk] = X0b

        def fe_layer(blk, k):
            b, half = blk // BLKB, blk % BLKB
            X0b = X0bs[blk]
            xb_prev = xprevs[blk]
            c = _cfg(k)
            idim = c["idim"]
            cols = LEAF_COLS >> k
            col0 = blk * cols
            vrep = fe2.tile([128, cols], U8, tag="vrep", name=f"vrep{blk}_{k}")
            vap = d[f"vec{k}"].ap()
            nc.sync.dma_start(out=vrep, in_=bass.AP(
                tensor=vap.tensor, offset=vap.offset + col0,
                ap=[[0, 128], [1, cols]]))
            ht = fe2.tile([128, c["nkt"], cols], F32, tag="fht", name=f"fht{blk}_{k}")
            for kt, r0, span, region, sub in _frags(k):
                nc.vector.scalar_tensor_tensor(
                    out=rr(ht[r0:r0 + span, kt, :], k),
                    in0=vrep[r0:r0 + span, :],
                    scalar=t_tc[k][r0:r0 + span, kt:kt + 1],
                    in1=xb_prev[r0:r0 + span, region::2],
                    op0=ISEQ, op1=MULT)
            bkt, br0 = divmod(c["bias0"], 128)
            nc.sync.dma_start(
                out=rr(ht[br0:br0 + NDIR, bkt, :], k),
                in_=rr(d[f"oh{k}"].ap()[:, col0:col0 + cols], k))
            xm = fe2.tile([128, cols], F32, tag=f"xm{k}", name=f"xm{blk}_{k}")
            nccs = max(1, cols // 512)
            ccw = min(512, cols)
            pss = [pp.tile([128, ccw], F32, tag="ps", name=f"fps{blk}_{k}_{i}")
                   for i in range(nccs)]
            for kt in range(c["nkt"]):
                rows = min(128, c["krows"] - kt * 128)
                wt = wp.tile([128, c["modim"]], F32, tag="wt")
                nc.sync.dma_start(out=rr(wt[:], k), in_=rr(d[f"wg{k}"].ap()[kt], k))
                for cc in range(nccs):
                    nc.tensor.matmul(
                        pss[cc][:], rr(wt[0:rows, 0:128], k),
                        rr(ht[0:rows, kt, cc * ccw:(cc + 1) * ccw], k),
                        start=(kt == 0), stop=(kt == c["nkt"] - 1))
            if k < S:
                for cc in range(nccs):
                    prelu_store(pss[cc][:], xm[:, cc * ccw:(cc + 1) * ccw],
                                alphas[k])
                kc = k + S
                scols = cols // 8
                nc.vector.tensor_copy(
                    rr(sg[kc][:, 0, b, half * scols:(half + 1) * scols], kc),
                    xm[:, ::8])
                xprevs[blk] = xm
            else:
                for cc in range(nccs):
                    prelu_store(pss[cc][:], rr(xm[:, cc * ccw:(cc + 1) * ccw], k),
                                alphas[k])
                ps2 = pp.tile([128, cols], F32, tag="ps", name=f"wps{blk}")
                for ci, (f, rows, src) in enumerate(c["segs"]):
                    wt2 = wsp.tile([128, c["modim"]], F32, tag="wst")
                    nc.sync.dma_start(out=rr(wt2[:], k),
                                      in_=rr(d[f"ws{k}"].ap()[ci], k))
                    rhs = (xm[0:rows, :] if src == "x"
                           else X0b[0:rows, ::8])
                    nc.tensor.matmul(ps2[:], rr(wt2[0:rows, 0:128], k),
                                     rr(rhs, k), start=(ci == 0),
                                     stop=(ci == len(c["segs"]) - 1))
                prelu_store(ps2[:],
                            X[S][:, 0, b, half * cols:(half + 1) * cols],
                            alphas[k], bias=t_bs[k][:, 0:1], nbias=t_nbs[k][:, 0:1])

        for pb in range(0, NBLK, 2):
            pair = (pb, pb + 1)
            for blk in pair:
                fe_leaf(blk)
            for k in range(1, S + 1):
                for blk in pair:
                    fe_layer(blk, k)
        fe_ctx.close()
        layerbuf = ctx.enter_context(tc.tile_pool(name="layerbuf", bufs=1))
        layer2 = ctx.enter_context(tc.tile_pool(name="layer2", bufs=2))
        htp = ctx.enter_context(tc.tile_pool(name="ht", bufs=2))
        wp = ctx.enter_context(tc.tile_pool(name="w", bufs=24))

        # sg for layer 6 comes from the completed X[3]
        nc.vector.tensor_copy(rr(sg[6][:, 0, :, :], 6), X[S][:, 0, :, ::8])

        # ---------------- back-end (feature-major): k = 4..TAIL0-1 ----------------
        for k in range(S + 1, TAIL0):
            c = _cfg(k)
            n, idim, odim = c["n"], c["idim"], c["odim"]
            cols = BPC * n
            vrep = layer2.tile([128, BPC, n], U8, tag="vrepB")
            vap = d[f"vec{k}"].ap()
            nc.sync.dma_start(out=vrep, in_=bass.AP(
                tensor=vap.tensor, offset=vap.offset,
                ap=[[0, 128], [n, BPC], [1, n]]))
            t_oh = layerbuf.tile([NDIR, BPC, n], F32, tag="ohB")
            nc.sync.dma_start(
                out=rr(t_oh[:], k),
                in_=rr(d[f"oh{k}"].ap().rearrange("p (b n) -> p b n", b=BPC), k))

            nccs = max(1, cols // 512)
            ccw = min(512, cols)
            pss = {(m, cc): pp.tile([128, ccw], F32, tag="ps", name=f"ps{k}_{m}_{cc}")
                   for m in range(c["mtiles"]) for cc in range(nccs)}
            xprev = X[k - 1]
            frags = _frags(k)
            nht = c["nkt"] - 1           # Htilde K-tiles (last tile is bias-only)
            gkt = max(1, 1024 // cols)   # K-tiles per ht group (~4KB)
            for kt0 in range(0, nht, gkt):
                kts = range(kt0, min(kt0 + gkt, nht))
                ht = htp.tile([128, len(kts), BPC, n], F32, tag="bht")
                for kt, r0, span, region, sub in frags:
                    if kt not in kts:
                        continue
                    nc.vector.scalar_tensor_tensor(
                        out=rr(ht[r0:r0 + span, kt - kt0, :, :], k),
                        in0=vrep[r0:r0 + span, :, :],
                        scalar=t_tc[k][r0:r0 + span, kt:kt + 1],
                        in1=xprev[r0:r0 + span, sub, :, region::2],
                        op0=ISEQ, op1=MULT)
                for kt in kts:
                    wt = wp.tile([128, c["modim"]], F32, tag="wt")
                    nc.sync.dma_start(out=rr(wt[:], k), in_=rr(d[f"wg{k}"].ap()[kt], k))
                    rhs2d = ht[:, kt - kt0, :, :].rearrange("p b n -> p (b n)")
                    for m in range(c["mtiles"]):
                        for cc in range(nccs):
                            nc.tensor.matmul(
                                pss[(m, cc)][:],
                                rr(wt[:, m * 128:(m + 1) * 128], k),
                                rr(rhs2d[:, cc * ccw:(cc + 1) * ccw], k),
                                start=(kt == 0), stop=False)
            wt = wp.tile([128, c["modim"]], F32, tag="wt")
            nc.sync.dma_start(out=rr(wt[:], k), in_=rr(d[f"wg{k}"].ap()[c["nkt"] - 1], k))
            oh2d = t_oh[:].rearrange("p b n -> p (b n)")
            for m in range(c["mtiles"]):
                for cc in range(nccs):
                    nc.tensor.matmul(
                        pss[(m, cc)][:], rr(wt[0:NDIR, m * 128:(m + 1) * 128], k),
                        rr(oh2d[:, cc * ccw:(cc + 1) * ccw], k),
                        start=False, stop=True)
            xm = layerbuf.tile([128, c["mtiles"], BPC, n], F32, tag="xmB")
            for m in range(c["mtiles"]):
                xm2d = xm[:, m, :, :].rearrange("p b n -> p (b n)")
                for cc in range(nccs):
                    prelu_store(pss[(m, cc)][:],
                                rr(xm2d[:, cc * ccw:(cc + 1) * ccw], k), alphas[k])
            pss2 = {(m, cc): pp.tile([128, ccw], F32, tag="ps", name=f"ps2_{k}_{m}_{cc}")
                    for m in range(c["mtiles"]) for cc in range(nccs)}
            for ci, (f, rows, src) in enumerate(c["segs"]):
                wt2 = wsp.tile([128, c["modim"]], F32, tag="wst")
                nc.sync.dma_start(out=rr(wt2[:], k), in_=rr(d[f"ws{k}"].ap()[ci], k))
                rhs2d = (xm[0:rows, f, :, :] if src == "x"
                         else sg[k][0:rows, f, :, :]).rearrange("p b n -> p (b n)")
                for m in range(c["mtiles"]):
                    for cc in range(nccs):
                        nc.tensor.matmul(
                            pss2[(m, cc)][:],
                            rr(wt2[0:rows, m * 128:(m + 1) * 128], k),
                            rr(rhs2d[:, cc * ccw:(cc + 1) * ccw], k),
                            start=(ci == 0), stop=(ci == len(c["segs"]) - 1))
            for m in range(c["mtiles"]):
                x2d = X[k][:, m, :, :].rearrange("p b n -> p (b n)")
                for cc in range(nccs):
                    prelu_store(pss2[(m, cc)][:], x2d[:, cc * ccw:(cc + 1) * ccw],
                                alphas[k], bias=t_bs[k][:, m:m + 1],
                                nbias=t_nbs[k][:, m:m + 1])
            kc = k + S
            if kc < NLAYERS:
                for f in range(max(1, odim // 128)):
                    nc.vector.tensor_copy(
                        rr(sg[kc][:, f, :, :], kc), X[k][:, f, :, ::8])

        # ---------------- tail (node-major, weight-streaming): k >= TAIL0 ----------------
        for k in range(TAIL0, NLAYERS):
            c = _cfg(k)
            n, idim, odim = c["n"], c["idim"], c["odim"]
            cols = BPC * n
            nsub = 2 * idim // 128          # ht K-tiles (8)
            xprev = X[k - 1]
            fprev = ODIMS[k - 1] // 128     # 4

            rrep = layer2.tile([128, BPC, n], U8, tag="rrepT", bufs=1)
            rap = d[f"rrep{k}"].ap()
            nc.sync.dma_start(out=rrep, in_=bass.AP(
                tensor=rap.tensor, offset=rap.offset,
                ap=[[0, 128], [n, BPC], [1, n]]))
            vcol = layer2.tile([128, 1], F32, tag="vcolT")
            nc.sync.dma_start(out=vcol[0:cols, :], in_=d[f"vcol{k}"].ap())
            mcol = layer2.tile([128, NDIR], F32, tag="mcolT")
            for t in range(NDIR):
                nc.vector.tensor_scalar(
                    out=mcol[0:cols, t:t + 1], in0=vcol[0:cols, :],
                    scalar1=float(t), scalar2=None, op0=ISEQ)

            # ht: [128, nsub, BPC, n]; rows of sub j = features of chunk j%4,
            # half j//4. top half: r=0 -> even child; bottom: r=0 -> odd.
            ht = htp.tile([128, nsub, BPC, n], F32, tag="tht")
            rrep4 = rrep[:].unsqueeze(1).broadcast_to([128, fprev, BPC, n])
            for half in range(2):
                ev = xprev[:, :, :, 0::2]
                od = xprev[:, :, :, 1::2]
                a_src = ev if half == 0 else od   # used when r == 0
                b_src = od if half == 0 else ev   # used when r == 1
                tmp1 = tp.tile([128, fprev, BPC, n], F32, tag="sel1", bufs=1)
                tmp2 = tp.tile([128, fprev, BPC, n], F32, tag="sel2", bufs=1)
                nc.vector.scalar_tensor_tensor(
                    out=tmp1[:], in0=rrep4, scalar=0.0, in1=a_src,
                    op0=ISEQ, op1=MULT)
                nc.vector.scalar_tensor_tensor(
                    out=tmp2[:], in0=rrep4, scalar=1.0, in1=b_src,
                    op0=ISEQ, op1=MULT)
                nc.vector.tensor_add(
                    rr(ht[:, half * fprev:(half + 1) * fprev, :, :], k),
                    tmp1[:], tmp2[:])

            # 7 direction matmuls: stationary ht tiles, streamed weights
            pst = {}
            for t in range(NDIR):
                pst[t] = pp.tile([128, odim], F32, tag="ps", name=f"pst{k}_{t}")
                for j in range(nsub):
                    wt = wp.tile([128, odim], F32, tag="wt")
                    nc.sync.dma_start(
                        out=rr(wt[:], k),
                        in_=rr(d[f"wgs{k}"].ap()[t][:, j * odim:(j + 1) * odim], k))
                    nc.tensor.matmul(
                        pst[t][0:cols, :],
                        rr(ht[:, j, :, :].rearrange("p b n -> p (b n)"), k),
                        rr(wt[:], k), start=(j == 0), stop=False)
                # bias: ones-row x bg[dmap[t]]
                wb = wsp.tile([1, odim], F32, tag="wbT", bufs=1)
                nc.sync.dma_start(out=rr(wb[:], k), in_=rr(d[f"bgm{k}"].ap()[t:t + 1, :], k))
                nc.tensor.matmul(pst[t][0:cols, :], rr(t_ones[:, 0:cols], k),
                                 rr(wb[:], k), start=False, stop=True)

            # select + prelu (node-major [cols, odim])
            # masked terms then pairwise tree-add (depth 3, ops independent)
            terms = []
            for t in range(NDIR):
                if t < NDIR - 1 and t % 2 == 0:
                    continue  # odd t handles the (t-1, t) pair below
                if t == NDIR - 1:
                    tt = layerbuf.tile([128, odim], F32, tag=f"selp{t // 2}",
                                       name=f"selp{k}_{t}")
                    nc.vector.tensor_scalar(
                        out=tt[0:cols, :], in0=pst[t][0:cols, :],
                        scalar1=mcol[0:cols, t:t + 1], scalar2=None, op0=MULT)
                else:
                    t0 = t - 1
                    h = layerbuf.tile([128, odim], F32, tag="selh",
                                      name=f"selh{k}_{t}")
                    nc.vector.tensor_scalar(
                        out=h[0:cols, :], in0=pst[t0][0:cols, :],
                        scalar1=mcol[0:cols, t0:t0 + 1], scalar2=None, op0=MULT)
                    tt = layerbuf.tile([128, odim], F32, tag=f"selp{t // 2}",
                                       name=f"selp{k}_{t}")
                    nc.vector.scalar_tensor_tensor(
                        out=tt[0:cols, :], in0=pst[t][0:cols, :],
                        scalar=mcol[0:cols, t:t + 1], in1=h[0:cols, :],
                        op0=MULT, op1=ADD)
                terms.append(tt)
            rnd = 0
            while len(terms) > 1:
                nxt = []
                for i in range(0, len(terms) - 1, 2):
                    o = layerbuf.tile([128, odim], F32,
                                      tag=f"selr{rnd}_{i // 2}",
                                      name=f"selr{k}_{rnd}_{i // 2}")
                    nc.vector.tensor_add(o[0:cols, :], terms[i][0:cols, :],
                                         terms[i + 1][0:cols, :])
                    nxt.append(o)
                if len(terms) % 2:
                    nxt.append(terms[-1])
                terms = nxt
                rnd += 1
            xm_nm = layerbuf.tile([128, odim], F32, tag="xmnmT")
            prelu_store(terms[0][0:cols, :], xm_nm[0:cols, :], alphas[k])

            # transpose xm to feature-major chunks for the Ws contraction
            xfm = layerbuf.tile([128, odim // 128, BPC, n], F32, tag="xfmT")
            for f in range(odim // 128):
                pstr = pp.tile([128, 128], F32, tag="ps", name=f"pstr{k}_{f}")
                nc.tensor.transpose(pstr[:, 0:cols],
                                    xm_nm[0:cols, f * 128:(f + 1) * 128],
                                    t_ident[0:cols, 0:cols])
                nc.vector.tensor_copy(
                    rr(xfm[:, f, :, :].rearrange("p b n -> p (b n)"), k),
                    pstr[:, 0:cols])

            # Ws stage: stationary = [xfm chunks | sg chunks], streamed Ws
            psw = pp.tile([128, odim], F32, tag="ps", name=f"psw{k}")
            nsegs = len(c["segs"])
            for ci, (f, rows, src) in enumerate(c["segs"]):
                wt2 = wsp.tile([128, odim], F32, tag="wst")
                nc.sync.dma_start(out=rr(wt2[:], k), in_=rr(d[f"ws{k}"].ap()[ci], k))
                lhs = (xfm[0:rows, f, :, :] if src == "x"
                       else sg[k][0:rows, f, :, :]).rearrange("p b n -> p (b n)")
                nc.tensor.matmul(psw[0:cols, :], rr(lhs, k),
                                 rr(wt2[0:rows, :], k),
                                 start=(ci == 0), stop=False)
            # bias bs via ones-row (bs blob is [modim,1] = [odim,1] here)
            wb2 = wsp.tile([1, odim], F32, tag="wb2T", bufs=1)
            nc.sync.dma_start(out=rr(wb2[:], k),
                              in_=rr(d[f"bs{k}"].ap().rearrange("o i -> i o"), k))
            nc.tensor.matmul(psw[0:cols, :], rr(t_ones[:, 0:cols], k),
                             rr(wb2[:], k), start=False, stop=True)

            x_nm = layerbuf.tile([128, odim], F32, tag="xnmT")
            prelu_store(psw[0:cols, :], x_nm[0:cols, :], alphas[k])

            # transpose back to feature-major X[k]
            for f in range(odim // 128):
                pstr = pp.tile([128, 128], F32, tag="ps", name=f"pstrb{k}_{f}")
                nc.tensor.transpose(pstr[:, 0:cols],
                                    x_nm[0:cols, f * 128:(f + 1) * 128],
                                    t_ident[0:cols, 0:cols])
                nc.vector.tensor_copy(
                    X[k][:, f, :, :].rearrange("p b n -> p (b n)"),
                    pstr[:, 0:cols])
            kc = k + S
            if kc < NLAYERS:
                for f in range(odim // 128):
                    nc.vector.tensor_copy(
                        rr(sg[kc][:, f, :, :], kc), X[k][:, f, :, ::8])

        for f in range(4):
            nc.sync.dma_start(out=d["out"].ap()[f * 128:(f + 1) * 128, :],
                              in_=X[NLAYERS - 1][:, f, :, 0])
    return d


def kernel(**inputs):
    points = np.asarray(inputs["points"], np.float32)
    vecs = [np.asarray(inputs[f"vec_{k}"]) for k in range(1, NLAYERS)]
    in_maps, alphas, _ = _host_prep(points, inputs["dmap"], inputs["drev"],
                                    vecs, inputs["params"])
    nc = bacc.Bacc()
    _build(nc, alphas)
    nc.compile()
    res = run_bass_kernel_spmd(nc, in_maps, list(range(NCORES)))
    out = np.zeros((B, 512), np.float32)
    for core in range(NCORES):
        out[core * BPC:(core + 1) * BPC] = res.results[core]["out"].T
    return out


# revision 39
# speedup vs baseline: 1.0907x; 1.0072x over previous
"""Trainium2 Bass kernel for nn_Encoder (tree GNN message passing).

Data-parallel over batch: 8 cores x 4 batch items. Feature-major layout
(features on partitions, (batch, node) on the free dim). The per-node
direction select is folded into the matmul: activations are block-stacked
into Htilde (K = 7*2*idim rows + 7 bias rows) whose rows are ordered
[even-sourced region | odd-sourced region | one-hot bias rows], so each
128-row K-tile is built with 1-2 full-width fused DVE ops
  out = (vec_rep == tcol) * child
with tcol a per-partition scalar column. Weights are host-reordered to
match (dmap/drev baked in); the per-node bias bg[dmap[t]] rides as 7 extra
K-rows whose rhs is the one-hot of vec. Outputs use M-replication (weights
tiled along M) so layers with odim<128 still fill all 128 partitions —
which also makes X replicated with period odim, which in turn makes every
partition range of X a valid child operand. PReLU = Relu(z) - a*Relu(-z)
(2 ACT passes + 1 fused DVE op). fp32r matmuls for leaf/k<=7, exact fp32
for k>=8 (same speed at their column counts).
"""

import numpy as np
from contextlib import ExitStack

import concourse.bass as bass
import concourse.bacc as bacc
import concourse.tile as tile
from concourse import mybir
from concourse.bass_utils import run_bass_kernel_spmd

# ----- problem constants (hardcoded per harness contract) -----
B = 32
N = 8192
NLAYERS = 14
S = 3                      # SAMPLE_LAYERS
NDIR = 7
NCORES = 8
BPC = B // NCORES          # batches per core = 4

ODIMS = [8]
for _ in range(1, NLAYERS):
    ODIMS.append(min(ODIMS[-1] * 2, 512))

F32 = mybir.dt.float32
F32R = mybir.dt.float32r
U8 = mybir.dt.uint8
RELU = mybir.ActivationFunctionType.Relu
MULT = mybir.AluOpType.mult
ADD = mybir.AluOpType.add
ISEQ = mybir.AluOpType.is_equal

# per-layer matmul dtype: True -> fp32r (fast), False -> fp32 (exact)
USE_F32R = {k: True for k in range(NLAYERS)}
TAIL0 = 8                 # first node-major (weight-streaming) layer

NBLK = 8                         # front-end blocks (half-batches)
LEAF_COLS = BPC * N // NBLK      # 4096 leaf cols per block


def _cfg(k):
    idim, odim = ODIMS[k - 1], ODIMS[k]
    n = N >> k
    rep = max(1, 128 // odim)
    modim = odim * rep
    mtiles = modim // 128
    reg_o = ((NDIR * idim + 31) // 32) * 32   # odd region start (32-aligned)
    bias0 = reg_o + NDIR * idim               # bias rows start
    krows = bias0 + NDIR                      # total K rows
    nkt = (krows + 127) // 128
    c = dict(k=k, idim=idim, odim=odim, n=n, rep=rep, modim=modim,
             mtiles=mtiles, krows=krows, nkt=nkt, reg_o=reg_o, bias0=bias0)
    if k >= S:
        sdim = ODIMS[k - S]
        fk = max(1, odim // 128)
        fs = max(1, sdim // 128)
        # Ws chain segments: x-chunks then sg-chunks, each its own K-tile
        segs = [(f, min(128, odim - f * 128), "x") for f in range(fk)]
        segs += [(f, min(128, sdim - f * 128), "s") for f in range(fs)]
        c.update(sdim=sdim, fk=fk, fs=fs, segs=segs)
    return c


def _frags(k):
    """STT build fragments per K-tile: (kt, r0, span, region, sub).
    Row layout: e-region [0, 7*idim), pad, o-region [reg_o, reg_o+7*idim),
    bias rows [bias0, bias0+7). Within a region: t = off // idim,
    feature = off % idim, sub = feature // 128. All fragment starts are
    32-aligned (hardware partition-offset granularity)."""
    c = _cfg(k)
    idim = c["idim"]
    # regions extended to cover padding/bias rows (tcol=99 there -> writes 0,
    # later overwritten by the bias DMA where applicable)
    regions = [(0, 0, c["reg_o"]), (1, c["reg_o"], c["nkt"] * 128)]
    out = []
    for kt in range((c["bias0"] + 127) // 128):
        lo = kt * 128
        hi = min(lo + 128, ((c["krows"] + 31) // 32) * 32)
        for region, ra, rb in regions:
            a, bnd = max(lo, ra), min(hi, rb)
            if a >= bnd:
                continue
            off = a - ra
            sub = (off % idim) // 128
            out.append((kt, a - lo, bnd - a, region, sub))
    return out




def _host_prep(points, dmap, drev, vecs, params):
    dmap = np.asarray(dmap).astype(np.int64)
    drev = np.asarray(drev).astype(np.int64)
    pl = params["leaf"]
    leafW = np.tile(np.asarray(pl["W"], np.float32), (1, 16))        # [3,128]
    leafb = np.tile(np.asarray(pl["b"], np.float32), 16)[:, None]    # [128,1]
    alphas = {0: float(pl["a"])}

    wg_blobs, ws_blobs, bs_blobs, tcols = {}, {}, {}, {}
    wgs_blobs, bgm_blobs = {}, {}
    for k in range(1, NLAYERS):
        c = _cfg(k)
        p = params["layers"][k - 1]
        alphas[k] = float(p["a"])
        Wg = np.asarray(p["Wg"], np.float32)   # [7, 2idim, odim]
        bg = np.asarray(p["bg"], np.float32)   # [7, odim]
        idim = c["idim"]
        blob = np.zeros((c["nkt"] * 128, c["modim"]), np.float32)
        tc_ = np.full((c["nkt"] * 128,), 99.0, np.float32)
        for r in range(2):
            for t in range(NDIR):
                hf = int(drev[t]) ^ r
                g0 = r * c["reg_o"] + t * idim
                blob[g0:g0 + idim, :] = np.tile(
                    Wg[dmap[t]][hf * idim:(hf + 1) * idim], (1, c["rep"]))
                tc_[g0:g0 + idim] = t
        for t in range(NDIR):
            blob[c["bias0"] + t, :] = np.tile(bg[dmap[t]], c["rep"])
        wg_blobs[k] = blob.reshape(c["nkt"], 128, c["modim"])
        tcols[k] = tc_.reshape(c["nkt"], 128).T.copy()   # [128, nkt]
        if k >= TAIL0:
            # streaming layout, partition-major: [7, 128, nsub*odim] so each
            # direction's weights DMA as one transfer with 16KB/partition runs
            wgs_blobs[k] = np.ascontiguousarray(
                np.stack([Wg[dmap[t]] for t in range(NDIR)]).reshape(
                    NDIR, 2 * idim // 128, 128, c["odim"]).transpose(0, 2, 1, 3)
                .reshape(NDIR, 128, (2 * idim // 128) * c["odim"]))
            bgm_blobs[k] = np.stack([bg[dmap[t]] for t in range(NDIR)])  # [7, odim]
        if k >= S:
            Ws = np.tile(np.asarray(p["Ws"], np.float32), (1, c["rep"]))
            bs = np.asarray(p["bs"], np.float32)
            blob2 = np.zeros((len(c["segs"]), 128, c["modim"]), np.float32)
            r0 = 0
            for i, (f, rows, _src) in enumerate(c["segs"]):
                blob2[i, :rows, :] = Ws[r0:r0 + rows]
                r0 += rows
            ws_blobs[k] = blob2
            bs_blobs[k] = np.tile(bs, c["rep"])[:, None]   # [modim, 1]

    in_maps = []
    for core in range(NCORES):
        bsl = slice(core * BPC, (core + 1) * BPC)
        m = {
            "pts": np.ascontiguousarray(
                np.asarray(points[bsl], np.float32).reshape(BPC * N, 3).T),
            "leafW": leafW, "leafb": leafb, "leafnb": -leafb,
        }
        for k in range(1, NLAYERS):
            c = _cfg(k)
            v = np.asarray(vecs[k - 1][bsl], np.int64).reshape(1, -1)
            if k < TAIL0:
                m[f"vec{k}"] = v.astype(np.uint8)
                oh = np.zeros((NDIR, BPC * c["n"]), np.float32)
                oh[v[0], np.arange(BPC * c["n"])] = 1.0
                m[f"oh{k}"] = oh
                m[f"wg{k}"] = wg_blobs[k]
                m[f"tc{k}"] = tcols[k]
            else:
                m[f"rrep{k}"] = drev[v].astype(np.uint8)        # [1, cols]
                m[f"vcol{k}"] = v.astype(np.float32).T          # [cols, 1]
                m[f"wgs{k}"] = wgs_blobs[k]
                m[f"bgm{k}"] = bgm_blobs[k]
            if k >= S:
                m[f"ws{k}"] = ws_blobs[k]
                m[f"bs{k}"] = bs_blobs[k]
                m[f"nbs{k}"] = -bs_blobs[k]
        in_maps.append(m)
    return in_maps, alphas, drev


def _build(nc, alphas):
    def rr(ap, k):
        return ap.bitcast(F32R) if USE_F32R[k] else ap

    d = {}
    d["pts"] = nc.declare_dram_parameter("pts", [3, BPC * N], F32, isOutput=False)
    d["leafW"] = nc.declare_dram_parameter("leafW", [3, 128], F32, isOutput=False)
    d["leafb"] = nc.declare_dram_parameter("leafb", [128, 1], F32, isOutput=False)
    d["leafnb"] = nc.declare_dram_parameter("leafnb", [128, 1], F32, isOutput=False)
    for k in range(1, NLAYERS):
        c = _cfg(k)
        cols = BPC * c["n"]
        if k < TAIL0:
            d[f"vec{k}"] = nc.declare_dram_parameter(f"vec{k}", [1, cols], U8, isOutput=False)
            d[f"oh{k}"] = nc.declare_dram_parameter(f"oh{k}", [NDIR, cols], F32, isOutput=False)
            d[f"wg{k}"] = nc.declare_dram_parameter(
                f"wg{k}", [c["nkt"], 128, c["modim"]], F32, isOutput=False)
            d[f"tc{k}"] = nc.declare_dram_parameter(f"tc{k}", [128, c["nkt"]], F32, isOutput=False)
        else:
            d[f"rrep{k}"] = nc.declare_dram_parameter(f"rrep{k}", [1, cols], U8, isOutput=False)
            d[f"vcol{k}"] = nc.declare_dram_parameter(f"vcol{k}", [cols, 1], F32, isOutput=False)
            d[f"wgs{k}"] = nc.declare_dram_parameter(
                f"wgs{k}", [NDIR, 128, (2 * c["idim"] // 128) * c["odim"]], F32, isOutput=False)
            d[f"bgm{k}"] = nc.declare_dram_parameter(f"bgm{k}", [NDIR, c["odim"]], F32, isOutput=False)
        if k >= S:
            d[f"ws{k}"] = nc.declare_dram_parameter(
                f"ws{k}", [len(c["segs"]), 128, c["modim"]], F32, isOutput=False)
            d[f"bs{k}"] = nc.declare_dram_parameter(f"bs{k}", [c["modim"], 1], F32, isOutput=False)
            d[f"nbs{k}"] = nc.declare_dram_parameter(f"nbs{k}", [c["modim"], 1], F32, isOutput=False)
    d["out"] = nc.declare_dram_parameter("out", [512, BPC], F32, isOutput=True)

    with tile.TileContext(nc) as tc, ExitStack() as ctx:
        persist = ctx.enter_context(tc.tile_pool(name="persist", bufs=1))
        wsp = ctx.enter_context(tc.tile_pool(name="ws", bufs=2))
        pp = ctx.enter_context(tc.tile_pool(name="ps", bufs=8, space="PSUM"))
        tp = ctx.enter_context(tc.tile_pool(name="tmp", bufs=2))

        t_leafW = persist.tile([3, 128], F32, tag="leafW")
        t_leafb = persist.tile([128, 1], F32, tag="leafb")
        t_leafnb = persist.tile([128, 1], F32, tag="leafnb")
        nc.sync.dma_start(out=rr(t_leafW[:], 0), in_=rr(d["leafW"].ap(), 0))
        nc.sync.dma_start(out=t_leafb, in_=d["leafb"].ap())
        nc.sync.dma_start(out=t_leafnb, in_=d["leafnb"].ap())
        t_bs, t_nbs, t_tc = {}, {}, {}
        for k in range(1, NLAYERS):
            c = _cfg(k)
            if k < TAIL0:
                t_tc[k] = persist.tile([128, c["nkt"]], F32, tag=f"tc{k}", name=f"tc{k}")
                nc.sync.dma_start(out=t_tc[k], in_=d[f"tc{k}"].ap())
            if k >= S:
                t_bs[k] = persist.tile([128, c["mtiles"]], F32, tag=f"bs{k}", name=f"bs{k}")
                t_nbs[k] = persist.tile([128, c["mtiles"]], F32, tag=f"nbs{k}", name=f"nbs{k}")
                nc.sync.dma_start(out=t_bs[k], in_=d[f"bs{k}"].ap().rearrange(
                    "(m p) o -> p (m o)", p=128))
                nc.sync.dma_start(out=t_nbs[k], in_=d[f"nbs{k}"].ap().rearrange(
                    "(m p) o -> p (m o)", p=128))

        from concourse.masks import make_identity
        t_ident = persist.tile([128, 128], F32, tag="ident")
        make_identity(nc, t_ident[:])
        t_ones = persist.tile([1, 128], F32, tag="ones")
        nc.vector.memset(t_ones[:], 1.0)

        X = {}
        for k in range(S, NLAYERS):
            c = _cfg(k)
            X[k] = persist.tile([128, max(1, c["odim"] // 128), BPC, c["n"]],
                                F32, tag=f"X{k}", name=f"X{k}")
        sg = {}
        for k in range(S + 1, NLAYERS):
            c = _cfg(k)
            sg[k] = persist.tile([128, c["fs"], BPC, c["n"]], F32, tag=f"sg{k}", name=f"sg{k}")

        def prelu_store(ps_ap, out_ap, alpha, bias=None, nbias=None, eng=None):
            # prelu(z+b) = pos + a*((z+b) - pos), pos = relu(z+b)
            # 1 ACT pass + 2 DVE ops
            shape = [ps_ap.partition_size(), ps_ap.free_size()]
            pos = tp.tile(shape, F32, tag="pos")
            neg = tp.tile(shape, F32, tag="neg")
            nc.scalar.activation(pos[:], ps_ap, RELU,
                                 bias=(bias if bias is not None else 0.0), scale=1.0)
            nc.vector.scalar_tensor_tensor(
                out=neg[:], in0=ps_ap,
                scalar=(bias if bias is not None else 0.0), in1=pos[:],
                op0=ADD, op1=mybir.AluOpType.subtract)
            nc.vector.scalar_tensor_tensor(
                out=out_ap, in0=neg[:], scalar=alpha, in1=pos[:],
                op0=MULT, op1=ADD)

        # ------------ front-end: leaf + k=1..3, pairs of blocks stage-major ------------
        # Front-end pools live only for this phase; their SBUF is returned
        # before the deep back-end weight-prefetch pool opens.
        fe_ctx = ExitStack()
        fe2 = fe_ctx.enter_context(tc.tile_pool(name="fe2", bufs=2))
        wp = fe_ctx.enter_context(tc.tile_pool(name="fwp", bufs=4))
        BLKB = NBLK // BPC               # blocks per batch item
        X0bs, xprevs = {}, {}

        def fe_leaf(blk):
            X0b = fe2.tile([128, LEAF_COLS], F32, tag="X0b", name=f"X0b{blk}")
            for cc in range(LEAF_COLS // 512):
                t_pts = fe2.tile([3, 512], F32, tag="pts", name=f"pts{blk}_{cc}")
                nc.sync.dma_start(
                    out=rr(t_pts[:], 0),
                    in_=rr(d["pts"].ap()[:, blk * LEAF_COLS + cc * 512:
                                         blk * LEAF_COLS + (cc + 1) * 512], 0))
                ps = pp.tile([128, 512], F32, tag="ps", name=f"lps{blk}_{cc}")
                nc.tensor.matmul(ps[:], rr(t_leafW[:], 0), rr(t_pts[:], 0),
                                 start=True, stop=True)
                prelu_store(ps[:], rr(X0b[:, cc * 512:(cc + 1) * 512], S),
                            alphas[0], bias=t_leafb[:], nbias=t_leafnb[:])
            X0bs[blk] = X0b
            xprevs[blk] = X0b

        def fe_layer(blk, k):
            b, half = blk // BLKB, blk % BLKB
            X0b = X0bs[blk]
            xb_prev = xprevs[blk]
            c = _cfg(k)
            idim = c["idim"]
            cols = LEAF_COLS >> k
            col0 = blk * cols
            vrep = fe2.tile([128, cols], U8, tag="vrep", name=f"vrep{blk}_{k}")
            vap = d[f"vec{k}"].ap()
            nc.sync.dma_start(out=vrep, in_=bass.AP(
                tensor=vap.tensor, offset=vap.offset + col0,
                ap=[[0, 128], [1, cols]]))
            ht = fe2.tile([128, c["nkt"], cols], F32, tag="fht", name=f"fht{blk}_{k}")
            for kt, r0, span, region, sub in _frags(k):
                nc.vector.scalar_tensor_tensor(
                    out=rr(ht[r0:r0 + span, kt, :], k),
                    in0=vrep[r0:r0 + span, :],
                    scalar=t_tc[k][r0:r0 + span, kt:kt + 1],
                    in1=xb_prev[r0:r0 + span, region::2],
                    op0=ISEQ, op1=MULT)
            bkt, br0 = divmod(c["bias0"], 128)
            nc.sync.dma_start(
                out=rr(ht[br0:br0 + NDIR, bkt, :], k),
                in_=rr(d[f"oh{k}"].ap()[:, col0:col0 + cols], k))
            xm = fe2.tile([128, cols], F32, tag=f"xm{k}", name=f"xm{blk}_{k}")
            nccs = max(1, cols // 512)
            ccw = min(512, cols)
            pss = [pp.tile([128, ccw], F32, tag="ps", name=f"fps{blk}_{k}_{i}")
                   for i in range(nccs)]
            for kt in range(c["nkt"]):
                rows = min(128, c["krows"] - kt * 128)
                wt = wp.tile([128, c["modim"]], F32, tag="wt")
                nc.sync.dma_start(out=rr(wt[:], k), in_=rr(d[f"wg{k}"].ap()[kt], k))
                for cc in range(nccs):
                    nc.tensor.matmul(
                        pss[cc][:], rr(wt[0:rows, 0:128], k),
                        rr(ht[0:rows, kt, cc * ccw:(cc + 1) * ccw], k),
                        start=(kt == 0), stop=(kt == c["nkt"] - 1))
            if k < S:
                for cc in range(nccs):
                    prelu_store(pss[cc][:], xm[:, cc * ccw:(cc + 1) * ccw],
                                alphas[k])
                kc = k + S
                scols = cols // 8
                nc.vector.tensor_copy(
                    rr(sg[kc][:, 0, b, half * scols:(half + 1) * scols], kc),
                    xm[:, ::8])
                xprevs[blk] = xm
            else:
                for cc in range(nccs):
                    prelu_store(pss[cc][:], rr(xm[:, cc * ccw:(cc + 1) * ccw], k),
                                alphas[k])
                ps2 = pp.tile([128, cols], F32, tag="ps", name=f"wps{blk}")
                for ci, (f, rows, src) in enumerate(c["segs"]):
                    wt2 = wsp.tile([128, c["modim"]], F32, tag="wst")
                    nc.sync.dma_start(out=rr(wt2[:], k),
                                      in_=rr(d[f"ws{k}"].ap()[ci], k))
                    rhs = (xm[0:rows, :] if src == "x"
                           else X0b[0:rows, ::8])
                    nc.tensor.matmul(ps2[:], rr(wt2[0:rows, 0:128], k),
                                     rr(rhs, k), start=(ci == 0),
                                     stop=(ci == len(c["segs"]) - 1))
                prelu_store(ps2[:],
                            X[S][:, 0, b, half * cols:(half + 1) * cols],
                            alphas[k], bias=t_bs[k][:, 0:1], nbias=t_nbs[k][:, 0:1])

        for pb in range(0, NBLK, 2):
            pair = (pb, pb + 1)
            for blk in pair:
                fe_leaf(blk)
            for k in range(1, S + 1):
                for blk in pair:
                    fe_layer(blk, k)
        fe_ctx.close()
        layerbuf = ctx.enter_context(tc.tile_pool(name="layerbuf", bufs=1))
        layer2 = ctx.enter_context(tc.tile_pool(name="layer2", bufs=2))
        htp = ctx.enter_context(tc.tile_pool(name="ht", bufs=2))
        wp = ctx.enter_context(tc.tile_pool(name="w", bufs=12))

        # sg for layer 6 comes from the completed X[3]
        nc.vector.tensor_copy(rr(sg[6][:, 0, :, :], 6), X[S][:, 0, :, ::8])

        # ---------------- back-end (feature-major): k = 4..TAIL0-1 ----------------
        for k in range(S + 1, TAIL0):
            c = _cfg(k)
            n, idim, odim = c["n"], c["idim"], c["odim"]
            cols = BPC * n
            vrep = layer2.tile([128, BPC, n], U8, tag="vrepB")
            vap = d[f"vec{k}"].ap()
            nc.sync.dma_start(out=vrep, in_=bass.AP(
                tensor=vap.tensor, offset=vap.offset,
                ap=[[0, 128], [n, BPC], [1, n]]))
            t_oh = layerbuf.tile([NDIR, BPC, n], F32, tag="ohB")
            nc.sync.dma_start(
                out=rr(t_oh[:], k),
                in_=rr(d[f"oh{k}"].ap().rearrange("p (b n) -> p b n", b=BPC), k))

            nccs = max(1, cols // 512)
            ccw = min(512, cols)
            pss = {(m, cc): pp.tile([128, ccw], F32, tag="ps", name=f"ps{k}_{m}_{cc}")
                   for m in range(c["mtiles"]) for cc in range(nccs)}
            xprev = X[k - 1]
            frags = _frags(k)
            nht = c["nkt"] - 1           # Htilde K-tiles (last tile is bias-only)
            gkt = max(1, 1024 // cols)   # K-tiles per ht group (~4KB)
            for kt0 in range(0, nht, gkt):
                kts = range(kt0, min(kt0 + gkt, nht))
                ht = htp.tile([128, len(kts), BPC, n], F32, tag="bht")
                for kt, r0, span, region, sub in frags:
                    if kt not in kts:
                        continue
                    nc.vector.scalar_tensor_tensor(
                        out=rr(ht[r0:r0 + span, kt - kt0, :, :], k),
                        in0=vrep[r0:r0 + span, :, :],
                        scalar=t_tc[k][r0:r0 + span, kt:kt + 1],
                        in1=xprev[r0:r0 + span, sub, :, region::2],
                        op0=ISEQ, op1=MULT)
                for kt in kts:
                    wt = wp.tile([128, c["modim"]], F32, tag="wt")
                    nc.sync.dma_start(out=rr(wt[:], k), in_=rr(d[f"wg{k}"].ap()[kt], k))
                    rhs2d = ht[:, kt - kt0, :, :].rearrange("p b n -> p (b n)")
                    for m in range(c["mtiles"]):
                        for cc in range(nccs):
                            nc.tensor.matmul(
                                pss[(m, cc)][:],
                                rr(wt[:, m * 128:(m + 1) * 128], k),
                                rr(rhs2d[:, cc * ccw:(cc + 1) * ccw], k),
                                start=(kt == 0), stop=False)
            wt = wp.tile([128, c["modim"]], F32, tag="wt")
            nc.sync.dma_start(out=rr(wt[:], k), in_=rr(d[f"wg{k}"].ap()[c["nkt"] - 1], k))
            oh2d = t_oh[:].rearrange("p b n -> p (b n)")
            for m in range(c["mtiles"]):
                for cc in range(nccs):
                    nc.tensor.matmul(
                        pss[(m, cc)][:], rr(wt[0:NDIR, m * 128:(m + 1) * 128], k),
                        rr(oh2d[:, cc * ccw:(cc + 1) * ccw], k),
                        start=False, stop=True)
            xm = layerbuf.tile([128, c["mtiles"], BPC, n], F32, tag="xmB")
            for m in range(c["mtiles"]):
                xm2d = xm[:, m, :, :].rearrange("p b n -> p (b n)")
                for cc in range(nccs):
                    prelu_store(pss[(m, cc)][:],
                                rr(xm2d[:, cc * ccw:(cc + 1) * ccw], k), alphas[k])
            pss2 = {(m, cc): pp.tile([128, ccw], F32, tag="ps", name=f"ps2_{k}_{m}_{cc}")
                    for m in range(c["mtiles"]) for cc in range(nccs)}
            for ci, (f, rows, src) in enumerate(c["segs"]):
                wt2 = wsp.tile([128, c["modim"]], F32, tag="wst")
                nc.sync.dma_start(out=rr(wt2[:], k), in_=rr(d[f"ws{k}"].ap()[ci], k))
                rhs2d = (xm[0:rows, f, :, :] if src == "x"
                         else sg[k][0:rows, f, :, :]).rearrange("p b n -> p (b n)")
                for m in range(c["mtiles"]):
                    for cc in range(nccs):
                        nc.tensor.matmul(
                            pss2[(m, cc)][:],
                            rr(wt2[0:rows, m * 128:(m + 1) * 128], k),
                            rr(rhs2d[:, cc * ccw:(cc + 1) * ccw], k),
                            start=(ci == 0), stop=(ci == len(c["segs"]) - 1))
            for m in range(c["mtiles"]):
                x2d = X[k][:, m, :, :].rearrange("p b n -> p (b n)")
                for cc in range(nccs):
                    prelu_store(pss2[(m, cc)][:], x2d[:, cc * ccw:(cc + 1) * ccw],
                                alphas[k], bias=t_bs[k][:, m:m + 1],
                                nbias=t_nbs[k][:, m:m + 1])
            kc = k + S
            if kc < NLAYERS:
                for f in range(max(1, odim // 128)):
                    nc.vector.tensor_copy(
                        rr(sg[kc][:, f, :, :], kc), X[k][:, f, :, ::8])

        # ---------------- tail (node-major, weight-streaming): k >= TAIL0 ----------------
        for k in range(TAIL0, NLAYERS):
            c = _cfg(k)
            n, idim, odim = c["n"], c["idim"], c["odim"]
            cols = BPC * n
            nsub = 2 * idim // 128          # ht K-tiles (8)
            xprev = X[k - 1]
            fprev = ODIMS[k - 1] // 128     # 4

            rrep = layer2.tile([128, BPC, n], U8, tag="rrepT", bufs=1)
            rap = d[f"rrep{k}"].ap()
            nc.sync.dma_start(out=rrep, in_=bass.AP(
                tensor=rap.tensor, offset=rap.offset,
                ap=[[0, 128], [n, BPC], [1, n]]))
            vcol = layer2.tile([128, 1], F32, tag="vcolT")
            nc.sync.dma_start(out=vcol[0:cols, :], in_=d[f"vcol{k}"].ap())
            mcol = layer2.tile([128, NDIR], F32, tag="mcolT")
            for t in range(NDIR):
                nc.vector.tensor_scalar(
                    out=mcol[0:cols, t:t + 1], in0=vcol[0:cols, :],
                    scalar1=float(t), scalar2=None, op0=ISEQ)

            # ht: [128, nsub, BPC, n]; rows of sub j = features of chunk j%4,
            # half j//4. top half: r=0 -> even child; bottom: r=0 -> odd.
            ht = htp.tile([128, nsub, BPC, n], F32, tag="tht")
            rrep4 = rrep[:].unsqueeze(1).broadcast_to([128, fprev, BPC, n])
            for half in range(2):
                ev = xprev[:, :, :, 0::2]
                od = xprev[:, :, :, 1::2]
                a_src = ev if half == 0 else od   # used when r == 0
                b_src = od if half == 0 else ev   # used when r == 1
                tmp1 = tp.tile([128, fprev, BPC, n], F32, tag="sel1", bufs=1)
                tmp2 = tp.tile([128, fprev, BPC, n], F32, tag="sel2", bufs=1)
                nc.vector.scalar_tensor_tensor(
                    out=tmp1[:], in0=rrep4, scalar=0.0, in1=a_src,
                    op0=ISEQ, op1=MULT)
                nc.vector.scalar_tensor_tensor(
                    out=tmp2[:], in0=rrep4, scalar=1.0, in1=b_src,
                    op0=ISEQ, op1=MULT)
                nc.vector.tensor_add(
                    rr(ht[:, half * fprev:(half + 1) * fprev, :, :], k),
                    tmp1[:], tmp2[:])

            # 7 direction matmuls: stationary ht tiles, streamed weights
            pst = {}
            for t in range(NDIR):
                pst[t] = pp.tile([128, odim], F32, tag="ps", name=f"pst{k}_{t}")
                for j0 in range(0, nsub, 2):
                    # 2 K-tiles per DMA: contiguous 4KB/partition runs in the
                    # partition-major blob, spread across queues per direction
                    wt2 = wp.tile([128, 2, odim], F32, tag="wt2", bufs=6)
                    nc.sync.dma_start(
                        out=rr(wt2[:].rearrange("p j o -> p (j o)"), k),
                        in_=rr(d[f"wgs{k}"].ap()[t][:, j0 * odim:(j0 + 2) * odim], k))
                    for j in (j0, j0 + 1):
                        nc.tensor.matmul(
                            pst[t][0:cols, :],
                            rr(ht[:, j, :, :].rearrange("p b n -> p (b n)"), k),
                            rr(wt2[:, j - j0, :], k), start=(j == 0), stop=False)
                # bias: ones-row x bg[dmap[t]]
                wb = wsp.tile([1, odim], F32, tag="wbT", bufs=1)
                nc.sync.dma_start(out=rr(wb[:], k), in_=rr(d[f"bgm{k}"].ap()[t:t + 1, :], k))
                nc.tensor.matmul(pst[t][0:cols, :], rr(t_ones[:, 0:cols], k),
                                 rr(wb[:], k), start=False, stop=True)

            # select + prelu (node-major [cols, odim])
            # masked terms then pairwise tree-add (depth 3, ops independent)
            terms = []
            for t in range(NDIR):
                if t < NDIR - 1 and t % 2 == 0:
                    continue  # odd t handles the (t-1, t) pair below
                if t == NDIR - 1:
                    tt = layerbuf.tile([128, odim], F32, tag=f"selp{t // 2}",
                                       name=f"selp{k}_{t}")
                    nc.vector.tensor_scalar(
                        out=tt[0:cols, :], in0=pst[t][0:cols, :],
                        scalar1=mcol[0:cols, t:t + 1], scalar2=None, op0=MULT)
                else:
                    t0 = t - 1
                    h = layerbuf.tile([128, odim], F32, tag="selh",
                                      name=f"selh{k}_{t}")
                    nc.vector.tensor_scalar(
                        out=h[0:cols, :], in0=pst[t0][0:cols, :],
                        scalar1=mcol[0:cols, t0:t0 + 1], scalar2=None, op0=MULT)
                    tt = layerbuf.tile([128, odim], F32, tag=f"selp{t // 2}",
                                       name=f"selp{k}_{t}")
                    nc.vector.scalar_tensor_tensor(
                        out=tt[0:cols, :], in0=pst[t][0:cols, :],
                        scalar=mcol[0:cols, t:t + 1], in1=h[0:cols, :],
                        op0=MULT, op1=ADD)
                terms.append(tt)
            rnd = 0
            while len(terms) > 1:
                nxt = []
                for i in range(0, len(terms) - 1, 2):
                    o = layerbuf.tile([128, odim], F32,
                                      tag=f"selr{rnd}_{i // 2}",
                                      name=f"selr{k}_{rnd}_{i // 2}")
                    nc.vector.tensor_add(o[0:cols, :], terms[i][0:cols, :],
                                         terms[i + 1][0:cols, :])
                    nxt.append(o)
                if len(terms) % 2:
                    nxt.append(terms[-1])
                terms = nxt
                rnd += 1
            xm_nm = layerbuf.tile([128, odim], F32, tag="xmnmT")
            prelu_store(terms[0][0:cols, :], xm_nm[0:cols, :], alphas[k])

            # transpose xm to feature-major chunks for the Ws contraction
            xfm = layerbuf.tile([128, odim // 128, BPC, n], F32, tag="xfmT")
            for f in range(odim // 128):
                pstr = pp.tile([128, 128], F32, tag="ps", name=f"pstr{k}_{f}")
                nc.tensor.transpose(pstr[:, 0:cols],
                                    xm_nm[0:cols, f * 128:(f + 1) * 128],
                                    t_ident[0:cols, 0:cols])
                nc.vector.tensor_copy(
                    rr(xfm[:, f, :, :].rearrange("p b n -> p (b n)"), k),
                    pstr[:, 0:cols])

            # Ws stage: stationary = [xfm chunks | sg chunks], streamed Ws
            psw = pp.tile([128, odim], F32, tag="ps", name=f"psw{k}")
            nsegs = len(c["segs"])
            for ci, (f, rows, src) in enumerate(c["segs"]):
                wt2 = wsp.tile([128, odim], F32, tag="wst")
                nc.sync.dma_start(out=rr(wt2[:], k), in_=rr(d[f"ws{k}"].ap()[ci], k))
                lhs = (xfm[0:rows, f, :, :] if src == "x"
                       else sg[k][0:rows, f, :, :]).rearrange("p b n -> p (b n)")
                nc.tensor.matmul(psw[0:cols, :], rr(lhs, k),
                                 rr(wt2[0:rows, :], k),
                                 start=(ci == 0), stop=False)
            # bias bs via ones-row (bs blob is [modim,1] = [odim,1] here)
            wb2 = wsp.tile([1, odim], F32, tag="wb2T", bufs=1)
            nc.sync.dma_start(out=rr(wb2[:], k),
                              in_=rr(d[f"bs{k}"].ap().rearrange("o i -> i o"), k))
            nc.tensor.matmul(psw[0:cols, :], rr(t_ones[:, 0:cols], k),
                             rr(wb2[:], k), start=False, stop=True)

            x_nm = layerbuf.tile([128, odim], F32, tag="xnmT")
            prelu_store(psw[0:cols, :], x_nm[0:cols, :], alphas[k])

            # transpose back to feature-major X[k]
            for f in range(odim // 128):
                pstr = pp.tile([128, 128], F32, tag="ps", name=f"pstrb{k}_{f}")
                nc.tensor.transpose(pstr[:, 0:cols],
                                    x_nm[0:cols, f * 128:(f + 1) * 128],
                                    t_ident[0:cols, 0:cols])
                nc.vector.tensor_copy(
                    X[k][:, f, :, :].rearrange("p b n -> p (b n)"),
                    pstr[:, 0:cols])
            kc = k + S
            if kc < NLAYERS:
                for f in range(odim // 128):
                    nc.vector.tensor_copy(
                        rr(sg[kc][:, f, :, :], kc), X[k][:, f, :, ::8])

        for f in range(4):
            nc.sync.dma_start(out=d["out"].ap()[f * 128:(f + 1) * 128, :],
                              in_=X[NLAYERS - 1][:, f, :, 0])
    return d


def kernel(**inputs):
    points = np.asarray(inputs["points"], np.float32)
    vecs = [np.asarray(inputs[f"vec_{k}"]) for k in range(1, NLAYERS)]
    in_maps, alphas, _ = _host_prep(points, inputs["dmap"], inputs["drev"],
                                    vecs, inputs["params"])
    nc = bacc.Bacc()
    _build(nc, alphas)
    nc.compile()
    res = run_bass_kernel_spmd(nc, in_maps, list(range(NCORES)))
    out = np.zeros((B, 512), np.float32)
    for core in range(NCORES):
        out[core * BPC:(core + 1) * BPC] = res.results[core]["out"].T
    return out
